# revision 61
# baseline (speedup 1.0000x reference)
"""DeformConv2d TRN2 kernel: build + host prep + SPMD runner.

Layout/algorithm summary (per core; 8 cores = 4 batches x 2 row-halves):
  - setup runs per row-block (a 1-subtile micro-block first, then 3/4/4/4):
    offset conv (3x3, 18 out ch) as 18 K-tile matmuls -> offsets ->
    PE-transpose to [n-part, 18] -> bilinear ints/fracs/weights on DVE.
    The micro-block makes the first subtile's indices ready ~12us in, and
    its 9 taps are gathered while the rest of setup still runs.
  - gather indices are built ON-CHIP in the wrapped [16-partition, s] layout
    dma_gather requires: one matmul per a with the constant selector
    R_a[p, m] = 1 iff p == a*16 + m%16 maps idxf [n-part, (st8 kk)] to
    psum[m, (st8 kk)] - the wrapped layout already replicated across all
    128 partitions; strided Act copies (f32->i16) assemble idxs16.
  - dma_gather on the PAIR table xpad2[r] = [pix r | pix r+72] with
    elem_size=1024, elem_step=512: ONE descriptor per (sample, tap) fetches
    all 4 bilinear corners (2KB).
  - combine: DVE builds 4 tiny diagonal matrices dg4[:, q, :] = idn * w_q
    (per-sample bilinear weight on the diagonal, [128,128] each); PE then
    computes gd_q^T @ diag(w_q) directly - the scaled transpose with the
    scaling folded into the matmul rhs (half the DVE elements of scaling
    the [128, 256] corners). The 4 corners accumulate in PSUM; Act copies
    PSUM -> sampled [ch, n] f16.
  - stage E runs quarter-major (h, g4, kk). Each quarter's 18-K-tile GEMM
    is mostly deferred into the next quarter's span (psG bufs=2 keeps two
    pso accumulator pairs alive): the deferred K-tiles read quarter-old
    sampled data, so they never stall the in-order PE queue and act as
    filler for its idle slots. The last quarter flushes inline (delay 2,
    then 1) and splits its PSUM->SBUF copies across Act/DVE so the drain
    after the final gather stays short. Bias is added at the PSUM read
    (per-partition bias column on Act / tensor_scalar_add on DVE).

Zero-padding of the table by 4 rows/cols emulates the reference's
valid-masking exactly for |excursion| <= 4; p is clamped to [0, 70.999] in
padded coords so larger offsets also read only zero-pad rows (-> exact 0).
"""

import sys

sys.path.insert(0, "/opt/trn_rl_repo")

import numpy as np

import bass_rust
import concourse.bass as bass
import concourse.bacc as bacc
import concourse.mybir as mybir
import concourse.tile as tile
from concourse import bass_utils
from concourse.tile_rust import add_dep_helper

P = 128
KK = 9
C = 256
H = W = 64
HO = 32          # rows per core (half image)
NS = HO * W      # samples per core = 2048
NT = NS // P     # 16 subtiles of 128 samples
PAD = 4
WP = 72          # padded width/height
NPIX = WP * WP   # 5184 pixels
TBL2 = 5113      # pair-table rows (idx <= 5110, fetch spans rows idx..idx+1)
F16 = mybir.dt.float16
F32 = mybir.dt.float32
I16 = mybir.dt.int16


def build(debug_outputs=False):
    nc = bacc.Bacc("TRN2", num_devices=8, debug=False)

    xpad2 = nc.dram_tensor("xpad2", [TBL2, 2 * C], F16, kind="ExternalInput")
    xchw = nc.dram_tensor("xchw", [2, P, 34 * WP], F16, kind="ExternalInput")
    wre = nc.dram_tensor("wre", [18, P, C], F16, kind="ExternalInput")
    owre = nc.dram_tensor("owre", [P, 18 * 18], F16, kind="ExternalInput")
    idn16d = nc.dram_tensor("idn16", [P, P], F16, kind="ExternalInput")
    # packed f32 constants: [obcol | bcols | basep4 | rsel | idn18-rows]
    cpkd = nc.dram_tensor("cpk", [P, 1 + 2 + NT * 18 + 8 * P + 18], F32, kind="ExternalInput")

    out = nc.dram_tensor("out", [C, NS], F16, kind="ExternalOutput")
    if debug_outputs:
        dbg_off = nc.dram_tensor("dbg_off", [18, NS], F32, kind="ExternalOutput")
        dbg_w4 = nc.dram_tensor("dbg_w4", [P, NT * KK * 4], F32, kind="ExternalOutput")
        dbg_idx = nc.dram_tensor("dbg_idx", [P, 2 * KK * 64], I16, kind="ExternalOutput")
        dbg_smp = nc.dram_tensor("dbg_smp", [P, 18 * NS], F16, kind="ExternalOutput")

    from contextlib import ExitStack

    AL = mybir.AluOpType

    with tile.TileContext(nc) as tc, ExitStack() as es:
        cst = es.enter_context(tc.tile_pool(name="cst", bufs=1))
        sb = es.enter_context(tc.tile_pool(name="sb", bufs=1))
        smpp = es.enter_context(tc.tile_pool(name="smp", bufs=3))
        gpool = es.enter_context(tc.tile_pool(name="gp", bufs=8))
        sclp = es.enter_context(tc.tile_pool(name="scl", bufs=4))
        otp = es.enter_context(tc.tile_pool(name="ot", bufs=4))
        psAB = ExitStack()
        psA = psAB.enter_context(tc.tile_pool(name="psA", bufs=2, space="PSUM"))
        psT = psAB.enter_context(tc.tile_pool(name="psT", bufs=2, space="PSUM"))

        # ---- constants, ordered so the offset-conv dependencies land first:
        # block 0 needs only xchw rows 0..4 + owre + obcol, so those lead.
        t_xchw = cst.tile([P, 2, 34 * WP], F16)
        nc.sync.dma_start(
            out=t_xchw[:, :, : 5 * WP],
            in_=xchw.ap().transpose([1, 0, 2])[:, :, : 5 * WP],
        )
        t_owre = cst.tile([P, 18, 18], F16)
        nc.sync.dma_start(out=t_owre[:], in_=owre.ap().rearrange("p (t d) -> p t d", d=18))
        t_cpk = cst.tile([P, 1 + 2 + NT * 18 + 8 * P + 18], F32)
        nc.sync.dma_start(out=t_cpk[:], in_=cpkd.ap())
        t_obcol = t_cpk[:, 0:1]
        t_bcols = t_cpk[:, 1:3]
        t_base = t_cpk[:, 3 : 3 + NT * 18]
        t_rsel = t_cpk[:, 3 + NT * 18 : 3 + NT * 18 + 8 * P].rearrange(
            "p (a m) -> p a m", m=P
        )
        t_idn18 = t_cpk[0:18, 3 + NT * 18 + 8 * P :]
        nc.sync.dma_start(
            out=t_xchw[:, :, 5 * WP : 12 * WP],
            in_=xchw.ap().transpose([1, 0, 2])[:, :, 5 * WP : 12 * WP],
        )
        t_idn16 = cst.tile([P, P], F16)
        nc.sync.dma_start(out=t_idn16[:], in_=idn16d.ap())
        nc.sync.dma_start(
            out=t_xchw[:, :, 12 * WP : 23 * WP],
            in_=xchw.ap().transpose([1, 0, 2])[:, :, 12 * WP : 23 * WP],
        )
        nc.sync.dma_start(
            out=t_xchw[:, :, 23 * WP :],
            in_=xchw.ap().transpose([1, 0, 2])[:, :, 23 * WP :],
        )

        # main-conv weights: needed only once the first gather lands
        t_wre = cst.tile([P, 18, C], F16)
        nc.sync.dma_start(out=t_wre[:], in_=wre.ap().transpose([1, 0, 2]))

        # PE p-state warmup with no DMA dependency (memset source): keeps PE
        # busy from ~t=0.3us so the first conv runs at mid/full clock.
        t_wu = cst.tile([64, 64], F16)
        nc.vector.memset(t_wu[:], 0.5)
        psW = psT.tile([P, P], F32, tag="psW")
        for i in range(40):
            nc.tensor.matmul(
                psW[0:64, 0:64], lhsT=t_wu[:], rhs=t_wu[:],
                start=(i == 0), stop=(i == 39),
            )

        # ---- per-group setup: offset conv -> transpose -> bilinear -> idx
        off_sb = sb.tile([P, NS], F32, tag="offsb")
        offT = sb.tile([P, NT, 18], F32, tag="offT")
        pP4 = sb.tile([P, NT, 18], F32, tag="pP4")
        pc = sb.tile([P, NT, 18], F32, tag="pc")
        i32 = sb.tile([P, NT, 18], mybir.dt.int32, tag="i32")
        ip0 = sb.tile([P, NT, 18], F32, tag="ip0")
        d0 = sb.tile([P, NT, 18], F32, tag="d0")
        msk = sb.tile([P, NT, 18], F32, tag="msk")
        ipart = sb.tile([P, NT, 18], F32, tag="ipart")
        frac = sb.tile([P, NT, 18], F32, tag="frac")
        omf = sb.tile([P, NT, 18], F32, tag="omf")
        w4 = sb.tile([P, NT, KK, 4], F32, tag="w4")
        idxf = sb.tile([P, NT, KK], F32, tag="idxf")
        idxs16 = []

        def ysl(t, sl):  # [128, n, 9] strided views (d = 2kk + {0:y, 1:x})
            v = t[:].rearrange("p s (k two) -> p s k two", two=2)
            return v[:, sl, :, 0]

        def xsl(t, sl):
            v = t[:].rearrange("p s (k two) -> p s k two", two=2)
            return v[:, sl, :, 1]

        xpad_src = bass.AP(xpad2, 0, [[2 * C, TBL2 - 1], [1, 4 * C]])
        # gather tiles for (h0, g4=0) taps 0..5: allocated upfront so block
        # 0's subtile can be gathered while the rest of setup still runs.
        # Capped below the gpool depth: a pre-gather whose tile needs a
        # recycled buffer would head-of-line-block the in-order Pool queue.
        NPRE = 6
        gd_pre = [
            gpool.tile([P, 4, 1024], F16, tag="gd", name=f"gd_0_0_{kk}")
            for kk in range(NPRE)
        ]

        setup_ctx = ExitStack()
        setup_ctx.enter_context(tc.high_priority(offset=100000))
        last_setup = {}
        last_bt = []
        # block 0 is a single 128-sample subtile: its indices are ready ~10us
        # before a full quarter's would be, so the first gathers start early
        blocks = [(0, 1), (1, 3), (4, 4), (8, 4), (12, 4)]
        for bi, (s0, n) in enumerate(blocks):
            h = s0 // 8
            sl = slice(s0, s0 + n)
            # stage A: offset conv for this block's rows (n*128 samples)
            ps = psA.tile([P, 512], F32, tag="psoff")
            for t in range(18):
                kk, ch = t // 2, t % 2
                ky, kx = kk // 3, kk % 3
                rhs = t_xchw[:, ch, :].rearrange("p (r w) -> p r w", w=WP)[
                    :, 2 * s0 + ky : 2 * s0 + ky + 2 * n, kx + 3 : kx + 3 + W
                ]
                i_cv = nc.tensor.matmul(
                    ps[0:18, : n * P],
                    lhsT=t_owre[:, t, :],
                    rhs=rhs,
                    start=(t == 0),
                    stop=(t == 17),
                )
                # keep PE stream in setup order: the next conv must not jump
                # ahead of this block's B-transposes (greedy scheduler would)
                if t == 0 and bi > 0:
                    add_dep_helper(i_cv.ins, last_bt[bi - 1].ins,
                                   reason="B-transposes before next conv")
            i_act = nc.scalar.add(
                off_sb[0:18, s0 * P : (s0 + n) * P], ps[0:18, : n * P], t_obcol[0:18, 0:1]
            )
            # stage B: transpose to offT [128, st, 18]; one merged copy
            pst4 = psA.tile([P, 4, 18], F32, tag="pstr", name=f"pst4_{bi}")
            for st in range(s0, s0 + n):
                i_bt = nc.tensor.transpose(
                    pst4[:, st - s0, 0:18],
                    in_=off_sb[0:18, st * P : (st + 1) * P],
                    identity=t_idn18,
                )
            nc.vector.tensor_copy(offT[:, sl, :], pst4[:, 0:n, :])
            last_bt.append(i_bt)
            # stage C: bilinear math on this block's slice [128, n*18]
            bsl = t_base.rearrange("p (s d) -> p s d", d=18)[:, sl, :]
            nc.vector.tensor_add(pP4[:, sl, :], offT[:, sl, :], bsl)
            nc.vector.tensor_scalar(pc[:, sl, :], pP4[:, sl, :], 0.0, 70.999, op0=AL.max, op1=AL.min)
            # floor robust to the f32->i32 cast mode: cast(pc - 0.5) is floor
            # under RNE (hw) but floor-1 for frac<0.5 under truncation
            # (interp); fix with d0 = pc - cast, msk = (d0 >= 1).
            nc.vector.tensor_scalar_add(i32[:, sl, :], pc[:, sl, :], -0.5)
            nc.vector.tensor_sub(d0[:, sl, :], pc[:, sl, :], i32[:, sl, :])
            nc.vector.tensor_scalar(msk[:, sl, :], d0[:, sl, :], 1.0, None, op0=AL.is_ge)
            nc.vector.tensor_add(ipart[:, sl, :], i32[:, sl, :], msk[:, sl, :])
            nc.vector.tensor_sub(frac[:, sl, :], d0[:, sl, :], msk[:, sl, :])
            nc.vector.tensor_scalar(omf[:, sl, :], frac[:, sl, :], -1.0, 1.0, op0=AL.mult, op1=AL.add)
            # w4 corner order of the pair-table fetch:
            # q0=(y0,x0), q1=(y1,x0), q2=(y0,x1), q3=(y1,x1)
            nc.vector.tensor_mul(w4[:, sl, :, 0], ysl(omf, sl), xsl(omf, sl))
            nc.vector.tensor_mul(w4[:, sl, :, 1], ysl(frac, sl), xsl(omf, sl))
            nc.vector.tensor_mul(w4[:, sl, :, 2], ysl(omf, sl), xsl(frac, sl))
            nc.vector.tensor_mul(w4[:, sl, :, 3], ysl(frac, sl), xsl(frac, sl))
            # idxf [128, n, 9]: pair-table row = 72*y0 + x0 (padded coords)
            nc.vector.scalar_tensor_tensor(
                idxf[:, sl, :], ysl(ipart, sl), 72.0, xsl(ipart, sl),
                op0=AL.mult, op1=AL.add,
            )

            # stage D: wrapped idx layout on-chip. Gather call (h, kk, g4)
            # slot i = st8*128 + p needs its idx at wrapped (r, s) = (i%16,
            # i//16) = (p%16, st8*8 + p//16), replicated over 16-partition
            # groups. One matmul per a with the constant selector R_a[p, m] =
            # 1 iff p == a*16 + m%16 yields psum[m, (st8 kk)] = idxf[a*16 +
            # m%16, (st8 kk)] - the wrapped layout, already replicated.
            if s0 % 8 == 0:
                # distinct tags: with a shared tag in this bufs=1 pool the
                # h=1 tile would REUSE h=0's buffer, making its writer wait
                # (WAR) on every h=0 gather — stalling the h transition and
                # poisoning the freed setup-PSUM banks' anti-deps
                idxs16.append(
                    sb.tile([P, KK, 8, 8], I16, tag="idxs16", name=f"idxs16_{h}")
                )
            ih = idxs16[h]
            sth = s0 - 8 * h
            psT8 = psT.tile([P, 8, 4 * KK], F32, tag="psT2", name=f"psT8_{bi}")
            for a in range(8):
                i_pe = nc.tensor.matmul(
                    psT8[:, a, : n * KK],
                    lhsT=t_rsel[:, a, :],
                    rhs=idxf[:, sl, :].rearrange("p a b -> p (a b)"),
                    start=True,
                    stop=True,
                )
            # one strided copy assembles the whole block's wrapped indices.
            # On Act, not DVE: the DVE queue gets stuffed with stage-E diag
            # builds, which would strand this copy (and the first h=1
            # gather behind it) tens of us out.
            i_dve = nc.scalar.copy(
                ih[:, :, sth : sth + n, :].transpose([0, 3, 2, 1]),
                psT8[:, :, : n * KK].rearrange("p a (s k) -> p a s k", k=KK),
            )
            last_setup = {"pe": i_pe, "dve": i_dve, "act": i_act}
            if bi == 0:
                # early gathers: block 0's 128 samples for taps 0..NPRE-1
                for kk in range(NPRE):
                    nc.gpsimd.dma_gather(
                        gd_pre[kk][:, 0:1, :],
                        xpad_src,
                        idxs16[0][:, kk, 0:1, :],
                        num_idxs=128,
                        num_idxs_reg=128,
                        elem_size=4 * C,
                        elem_step=2 * C,
                    )

        setup_ctx.close()
        tc.cur_priority += 500000  # push stage E far behind setup in the ready heap
        if debug_outputs:
            nc.sync.dma_start(out=dbg_off.ap(), in_=off_sb[0:18, :])
            nc.sync.dma_start(out=dbg_w4.ap(), in_=w4[:].rearrange("p a b c -> p (a b c)"))
            for h in range(2):
                nc.sync.dma_start(
                    out=dbg_idx.ap().rearrange("p (h n) -> p h n", h=2)[:, h, :],
                    in_=idxs16[h][:].rearrange("p a b c -> p (a b c)"),
                )

        psAB.close()  # free setup PSUM banks
        psE = es.enter_context(tc.tile_pool(name="psE", bufs=2, space="PSUM"))
        psG = es.enter_context(tc.tile_pool(name="psG", bufs=2, space="PSUM"))

        # ---- stage E: gather + scale + PSUM-accumulate transpose + GEMM.
        # Quarter-major order (h, g4, kk). Each quarter's GEMM is DEFERRED
        # into the next quarter's span (psG bufs=2 keeps both pso sets
        # alive): the deferred flushes read quarter-old sampled data, so
        # they never stall the in-order PE queue and instead fill every PE
        # idle slot. The last quarter flushes inline so nothing is left for
        # the drain but the final taps.
        def mk_flush(pso_, sampled_):
            def flush_gemm(dk):
                for ch in range(2):
                    t = dk * 2 + ch
                    for oh in range(2):
                        nc.tensor.matmul(
                            pso_[oh][:],
                            lhsT=t_wre[:, t, oh * P : (oh + 1) * P],
                            rhs=sampled_[:, t, :],
                            start=(t == 0),
                            stop=(t == 17),
                        )
            return flush_gemm

        def emit_out(pso_, h_, g4_, last_q_):
            for oh in range(2):
                ot = otp.tile([P, 512], F16, tag="ot", name=f"ot_{h_}_{g4_}_{oh}")
                # bias added at the PSUM read (per-partition col); the final
                # quarter's pair splits across Act/DVE so the copies overlap
                # in the drain
                if last_q_ and oh == 1:
                    nc.vector.tensor_scalar_add(ot[:], pso_[oh][:], t_bcols[:, 1:2])
                else:
                    nc.scalar.activation(
                        ot[:],
                        pso_[oh][:],
                        mybir.ActivationFunctionType.Identity,
                        bias=t_bcols[:, oh : oh + 1],
                    )
                nc.sync.dma_start(
                    out=bass.AP(
                        out, oh * P * NS + h_ * 1024 + g4_ * 512, [[NS, P], [1, 512]]
                    ),
                    in_=ot[:],
                )

        prev = None  # (pend, flush_fn, pso, h, g4) of the previous quarter
        for h in range(2):
            for g4 in range(2):
                sampled = smpp.tile(
                    [P, 18, 512], F16, tag="sampled", name=f"smp_{h}_{g4}"
                )
                pso = [
                    psG.tile([P, 512], F32, tag=f"pso{oh}", name=f"pso{oh}_{h}_{g4}")
                    for oh in range(2)
                ]
                pend = []
                flush_gemm = mk_flush(pso, sampled)

                last_q = h == 1 and g4 == 1
                first_q = h == 0 and g4 == 0
                for kk in range(KK):
                    if first_q and kk < NPRE:
                        # subtile 0 was gathered during setup; fetch st 1..3
                        gd = gd_pre[kk]
                        nc.gpsimd.dma_gather(
                            gd[:, 1:4, :],
                            xpad_src,
                            idxs16[h][:, kk, 1:4, :],
                            num_idxs=384,
                            num_idxs_reg=384,
                            elem_size=4 * C,
                            elem_step=2 * C,
                        )
                    elif last_q and kk >= KK - 5:
                        # split the final gathers so the per-subtile combines
                        # overlap the remaining sub-transfers (shrinks the
                        # drain tail)
                        gd = gpool.tile([P, 4, 1024], F16, tag="gd", name=f"gd_{h}_{g4}_{kk}")
                        for i4g in range(4):
                            nc.gpsimd.dma_gather(
                                gd[:, i4g : i4g + 1, :],
                                xpad_src,
                                idxs16[h][:, kk, g4 * 4 + i4g : g4 * 4 + i4g + 1, :],
                                num_idxs=128,
                                num_idxs_reg=128,
                                elem_size=4 * C,
                                elem_step=2 * C,
                            )
                    else:
                        gd = gpool.tile([P, 4, 1024], F16, tag="gd", name=f"gd_{h}_{g4}_{kk}")
                        nc.gpsimd.dma_gather(
                            gd[:],
                            xpad_src,
                            idxs16[h][:, kk, g4 * 4 : (g4 + 1) * 4, :],
                            num_idxs=512,
                            num_idxs_reg=512,
                            elem_size=4 * C,
                            elem_step=2 * C,
                        )
                    ptile = [
                        psE.tile(
                            [P, 512], F32, tag=f"pt{ch}", name=f"pt{ch}_{h}_{g4}_{kk}"
                        )
                        for ch in range(2)
                    ]
                    for i4 in range(4):
                        st8 = g4 * 4 + i4
                        st = h * 8 + st8
                        # diag trick: dg4[:, q, :] = idn * w_q (per-sample
                        # diagonal); PE computes gd_q^T @ diag(w_q), i.e. the
                        # scaled transpose, with the scaling folded into the
                        # matmul rhs. Half the DVE elements of scaling the
                        # [128, 256] corners directly.
                        dg4 = sclp.tile([P, 4, P], F16, tag="dg4")
                        for q in range(4):
                            nc.vector.tensor_scalar_mul(
                                dg4[:, q, :],
                                t_idn16[:],
                                w4[:, st, kk, q : q + 1],
                            )
                        for ch in range(2):
                            for q in range(4):
                                nc.tensor.matmul(
                                    ptile[ch][:, i4 * P : (i4 + 1) * P],
                                    lhsT=gd[:, i4, q * C + ch * P : q * C + (ch + 1) * P],
                                    rhs=dg4[:, q, :],
                                    start=(q == 0),
                                    stop=(q == 3),
                                )
                    for ch in range(2):
                        t = kk * 2 + ch
                        # drain phase of the last quarter: split the copies
                        # across Act/DVE so the PSUM->sampled stage doesn't
                        # serialize the in-order queues
                        if last_q and kk >= 4 and ch == 1:
                            nc.vector.tensor_copy(sampled[:, t, :], ptile[ch][:])
                        else:
                            nc.scalar.copy(sampled[:, t, :], ptile[ch][:])
                    pend.append(kk)
                    if last_q:
                        # drain the leftover previous-quarter flushes, then
                        # stream this quarter's inline with delay 2 (1 for
                        # the final taps)
                        if prev is not None and prev[0]:
                            prev[1](prev[0].pop(0))
                            if kk >= 5 and prev[0]:
                                prev[1](prev[0].pop(0))
                        depth = 1 if kk >= KK - 2 else 2
                        while len(pend) > depth:
                            flush_gemm(pend.pop(0))
                    elif prev is not None and prev[0]:
                        # deferred GEMM of the previous quarter: one K-tile
                        # pair per tap; its data is a whole quarter old, so
                        # these never stall the PE queue — they fill its
                        # idle slots
                        prev[1](prev[0].pop(0))
                if prev is not None:
                    while prev[0]:
                        prev[1](prev[0].pop(0))
                    emit_out(prev[2], prev[3], prev[4], False)
                if last_q:
                    for dk in pend:
                        flush_gemm(dk)
                    emit_out(pso, h, g4, True)
                prev = (pend, flush_gemm, pso, h, g4)
                if debug_outputs:
                    nc.sync.dma_start(
                        out=dbg_smp.ap().rearrange(
                            "p (t q n) -> p t q n", q=4, n=512
                        )[:, :, h * 2 + g4, :],
                        in_=sampled[:],
                    )

    nc.compile()
    return nc


def host_prep(x, weight, bias, offset_w, offset_b):
    """Returns (in_maps list of 8 dicts, assemble fn)."""
    B = x.shape[0]
    xp = np.zeros((B, WP, WP, C), np.float16)
    xp[:, PAD : PAD + H, PAD : PAD + W, :] = x.transpose(0, 2, 3, 1)
    # pair table: row r = [pixel r | pixel r+72] so one 2KB fetch at rows
    # (r, r+1) yields all 4 bilinear corners.
    xpad2_b = []
    for b in range(B):
        flat = xp[b].reshape(NPIX, C)
        t2 = np.zeros((TBL2, 2 * C), np.float16)
        t2[: TBL2 - 1, 0:C] = flat[: TBL2 - 1]
        t2[: TBL2 - 1, C : 2 * C] = flat[72 : TBL2 - 1 + 72]
        xpad2_b.append(t2)
    # c-major padded image for the offset conv, per (b, hh): rows 32h+3 .. +37
    xcp = xp.transpose(0, 3, 1, 2).reshape(B, 2, P, WP, WP)  # [b, grp, 128, 72, 72]
    wre = np.ascontiguousarray(
        weight.reshape(C, 2, P, 3, 3).transpose(3, 4, 1, 2, 0).reshape(KK * 2, P, C)
    ).astype(np.float16)
    # t = kk*2 + ch ; value = offset_w[o, ch*128+i, ky, kx]; packed [P, 18*18]
    owre = np.ascontiguousarray(
        offset_w.reshape(18, 2, P, 3, 3).transpose(2, 3, 4, 1, 0).reshape(P, 18 * 18)
    ).astype(np.float16)
    idn16 = np.eye(P, dtype=np.float16)
    obcol = np.zeros((P, 1), np.float32)
    obcol[:18, 0] = offset_b
    # selector for the wrapped-idx matmuls: rsel[p, a, m] = 1 iff p == a*16 + m%16
    rsel = np.zeros((P, 8, P), np.float32)
    for a in range(8):
        for m in range(P):
            rsel[a * 16 + m % 16, a, m] = 1.0
    rsel = rsel.reshape(P, 8 * P)
    bcols = np.asarray(bias, np.float32).reshape(2, P).T.copy()  # [128, 2]

    base_all = []
    for hh in range(2):
        base = np.zeros((P, NT, 18), np.float32)
        p = np.arange(P)
        for st in range(NT):
            n = st * P + p
            ho = 32 * hh + n // W
            wo = n % W
            for kk in range(KK):
                ky, kx = kk // 3, kk % 3
                base[:, st, 2 * kk + 0] = ky + ho - 1 + PAD
                base[:, st, 2 * kk + 1] = kx + wo - 1 + PAD
        base_all.append(base.reshape(P, NT * 18))

    # packed f32 constants: [obcol | bcols | basep4 | rsel | idn18-rows]
    cpk_all = []
    for hh in range(2):
        cpk = np.zeros((P, 1 + 2 + NT * 18 + 8 * P + 18), np.float32)
        cpk[:, 0:1] = obcol
        cpk[:, 1:3] = bcols
        cpk[:, 3 : 3 + NT * 18] = base_all[hh]
        cpk[:, 3 + NT * 18 : 3 + NT * 18 + 8 * P] = rsel
        cpk[0:18, 3 + NT * 18 + 8 * P :] = np.eye(18, dtype=np.float32)
        cpk_all.append(cpk)

    in_maps = []
    for core in range(8):
        b, hh = core // 2, core % 2
        in_maps.append(
            {
                "xpad2": xpad2_b[b],
                "xchw": np.ascontiguousarray(
                    xcp[b, :, :, 32 * hh + 3 : 32 * hh + 37, :].reshape(2, P, 34 * WP)
                ),
                "wre": wre,
                "owre": owre,
                "cpk": cpk_all[hh],
                "idn16": idn16,
            }
        )

    def assemble(results):
        y = np.empty((B, C, H, W), np.float32)
        for core in range(8):
            b, hh = core // 2, core % 2
            y[b, :, 32 * hh : 32 * (hh + 1), :] = (
                results[core]["out"].astype(np.float32).reshape(C, HO, W)
            )
        return y

    return in_maps, assemble


_CACHE = {}


def _maybe_reset_devices():
    # Clear any wedged accelerator state left by a previous crashed run.
    try:
        import ctypes
        import jax

        jax.devices()
        lib = ctypes.CDLL("/opt/axon/libaxon_pjrt.so")
        if hasattr(lib, "axon_reset"):
            lib.axon_reset.restype = ctypes.c_int64
            lib.axon_reset()
    except Exception:
        pass


def kernel(x, weight, bias, offset_w, offset_b, trace=False):
    if "nc" not in _CACHE:
        _maybe_reset_devices()
        _CACHE["nc"] = build()
    nc = _CACHE["nc"]
    in_maps, assemble = host_prep(
        np.asarray(x), np.asarray(weight), np.asarray(bias),
        np.asarray(offset_w), np.asarray(offset_b),
    )
    res = bass_utils.run_bass_kernel_spmd(
        nc, in_maps, core_ids=list(range(8)), trace=trace
    )
    out = assemble(res.results)
    _CACHE["last_exec_time_ns"] = res.exec_time_ns
    return out



# revision 62
# speedup vs baseline: 1.0011x; 1.0011x over previous
"""DeformConv2d TRN2 kernel: build + host prep + SPMD runner.

Layout/algorithm summary (per core; 8 cores = 4 batches x 2 row-halves):
  - setup runs per row-block (a 1-subtile micro-block first, then 3/4/4/4):
    offset conv (3x3, 18 out ch) as 18 K-tile matmuls -> offsets ->
    PE-transpose to [n-part, 18] -> bilinear ints/fracs/weights on DVE.
    The micro-block makes the first subtile's indices ready ~12us in, and
    its 9 taps are gathered while the rest of setup still runs.
  - gather indices are built ON-CHIP in the wrapped [16-partition, s] layout
    dma_gather requires: one matmul per a with the constant selector
    R_a[p, m] = 1 iff p == a*16 + m%16 maps idxf [n-part, (st8 kk)] to
    psum[m, (st8 kk)] - the wrapped layout already replicated across all
    128 partitions; strided Act copies (f32->i16) assemble idxs16.
  - dma_gather on the PAIR table xpad2[r] = [pix r | pix r+72] with
    elem_size=1024, elem_step=512: ONE descriptor per (sample, tap) fetches
    all 4 bilinear corners (2KB).
  - combine: DVE builds 4 tiny diagonal matrices dg4[:, q, :] = idn * w_q
    (per-sample bilinear weight on the diagonal, [128,128] each); PE then
    computes gd_q^T @ diag(w_q) directly - the scaled transpose with the
    scaling folded into the matmul rhs (half the DVE elements of scaling
    the [128, 256] corners). The 4 corners accumulate in PSUM; Act copies
    PSUM -> sampled [ch, n] f16.
  - stage E runs quarter-major (h, g4, kk). Each quarter's 18-K-tile GEMM
    is mostly deferred into the next quarter's span (psG bufs=2 keeps two
    pso accumulator pairs alive): the deferred K-tiles read quarter-old
    sampled data, so they never stall the in-order PE queue and act as
    filler for its idle slots. The last quarter flushes inline (delay 2,
    then 1) and splits its PSUM->SBUF copies across Act/DVE so the drain
    after the final gather stays short. Bias is added at the PSUM read
    (per-partition bias column on Act / tensor_scalar_add on DVE).

Zero-padding of the table by 4 rows/cols emulates the reference's
valid-masking exactly for |excursion| <= 4; p is clamped to [0, 70.999] in
padded coords so larger offsets also read only zero-pad rows (-> exact 0).
"""

import sys

sys.path.insert(0, "/opt/trn_rl_repo")

import numpy as np

import bass_rust
import concourse.bass as bass
import concourse.bacc as bacc
import concourse.mybir as mybir
import concourse.tile as tile
from concourse import bass_utils
from concourse.tile_rust import add_dep_helper

P = 128
KK = 9
C = 256
H = W = 64
HO = 32          # rows per core (half image)
NS = HO * W      # samples per core = 2048
NT = NS // P     # 16 subtiles of 128 samples
PAD = 4
WP = 72          # padded width/height
NPIX = WP * WP   # 5184 pixels
TBL2 = 5113      # pair-table rows (idx <= 5110, fetch spans rows idx..idx+1)
F16 = mybir.dt.float16
F32 = mybir.dt.float32
I16 = mybir.dt.int16


def build(debug_outputs=False):
    nc = bacc.Bacc("TRN2", num_devices=8, debug=False)

    xpad2 = nc.dram_tensor("xpad2", [TBL2, 2 * C], F16, kind="ExternalInput")
    xchw = nc.dram_tensor("xchw", [2, P, 34 * WP], F16, kind="ExternalInput")
    wre = nc.dram_tensor("wre", [18, P, C], F16, kind="ExternalInput")
    owre = nc.dram_tensor("owre", [P, 18 * 18], F16, kind="ExternalInput")
    idn16d = nc.dram_tensor("idn16", [P, P], F16, kind="ExternalInput")
    # packed f32 constants: [obcol | bcols | basep4 | rsel | idn18-rows]
    cpkd = nc.dram_tensor("cpk", [P, 1 + 2 + NT * 18 + 8 * P + 18], F32, kind="ExternalInput")

    out = nc.dram_tensor("out", [C, NS], F16, kind="ExternalOutput")
    if debug_outputs:
        dbg_off = nc.dram_tensor("dbg_off", [18, NS], F32, kind="ExternalOutput")
        dbg_w4 = nc.dram_tensor("dbg_w4", [P, NT * KK * 4], F32, kind="ExternalOutput")
        dbg_idx = nc.dram_tensor("dbg_idx", [P, 2 * KK * 64], I16, kind="ExternalOutput")
        dbg_smp = nc.dram_tensor("dbg_smp", [P, 18 * NS], F16, kind="ExternalOutput")

    from contextlib import ExitStack

    AL = mybir.AluOpType

    with tile.TileContext(nc) as tc, ExitStack() as es:
        cst = es.enter_context(tc.tile_pool(name="cst", bufs=1))
        sb = es.enter_context(tc.tile_pool(name="sb", bufs=1))
        smpp = es.enter_context(tc.tile_pool(name="smp", bufs=3))
        gpool = es.enter_context(tc.tile_pool(name="gp", bufs=8))
        sclp = es.enter_context(tc.tile_pool(name="scl", bufs=4))
        otp = es.enter_context(tc.tile_pool(name="ot", bufs=4))
        psAB = ExitStack()
        psA = psAB.enter_context(tc.tile_pool(name="psA", bufs=2, space="PSUM"))
        psT = psAB.enter_context(tc.tile_pool(name="psT", bufs=2, space="PSUM"))

        # ---- constants, ordered so the offset-conv dependencies land first:
        # block 0 needs only xchw rows 0..4 + owre + obcol, so those lead.
        t_xchw = cst.tile([P, 2, 34 * WP], F16)
        nc.sync.dma_start(
            out=t_xchw[:, :, : 5 * WP],
            in_=xchw.ap().transpose([1, 0, 2])[:, :, : 5 * WP],
        )
        t_owre = cst.tile([P, 18, 18], F16)
        nc.sync.dma_start(out=t_owre[:], in_=owre.ap().rearrange("p (t d) -> p t d", d=18))
        t_cpk = cst.tile([P, 1 + 2 + NT * 18 + 8 * P + 18], F32)
        # first 3 cols (obcol/bcols) early: block 0's act-add needs obcol
        nc.sync.dma_start(out=t_cpk[:, 0:3], in_=cpkd.ap()[:, 0:3])
        t_obcol = t_cpk[:, 0:1]
        t_bcols = t_cpk[:, 1:3]
        t_base = t_cpk[:, 3 : 3 + NT * 18]
        t_rsel = t_cpk[:, 3 + NT * 18 : 3 + NT * 18 + 8 * P].rearrange(
            "p (a m) -> p a m", m=P
        )
        t_idn18 = t_cpk[0:18, 3 + NT * 18 + 8 * P :]
        nc.sync.dma_start(
            out=t_xchw[:, :, 5 * WP : 12 * WP],
            in_=xchw.ap().transpose([1, 0, 2])[:, :, 5 * WP : 12 * WP],
        )
        nc.sync.dma_start(out=t_cpk[:, 3:], in_=cpkd.ap()[:, 3:])
        t_idn16 = cst.tile([P, P], F16)
        nc.sync.dma_start(out=t_idn16[:], in_=idn16d.ap())
        nc.sync.dma_start(
            out=t_xchw[:, :, 12 * WP : 23 * WP],
            in_=xchw.ap().transpose([1, 0, 2])[:, :, 12 * WP : 23 * WP],
        )
        nc.sync.dma_start(
            out=t_xchw[:, :, 23 * WP :],
            in_=xchw.ap().transpose([1, 0, 2])[:, :, 23 * WP :],
        )

        # main-conv weights: needed only once the first gather lands
        t_wre = cst.tile([P, 18, C], F16)
        nc.sync.dma_start(out=t_wre[:], in_=wre.ap().transpose([1, 0, 2]))

        # PE p-state warmup with no DMA dependency (memset source): keeps PE
        # busy from ~t=0.3us so the first conv runs at mid/full clock.
        t_wu = cst.tile([64, 64], F16)
        nc.vector.memset(t_wu[:], 0.5)
        psW = psT.tile([P, P], F32, tag="psW")
        for i in range(40):
            nc.tensor.matmul(
                psW[0:64, 0:64], lhsT=t_wu[:], rhs=t_wu[:],
                start=(i == 0), stop=(i == 39),
            )

        # ---- per-group setup: offset conv -> transpose -> bilinear -> idx
        off_sb = sb.tile([P, NS], F32, tag="offsb")
        offT = sb.tile([P, NT, 18], F32, tag="offT")
        pP4 = sb.tile([P, NT, 18], F32, tag="pP4")
        pc = sb.tile([P, NT, 18], F32, tag="pc")
        i32 = sb.tile([P, NT, 18], mybir.dt.int32, tag="i32")
        ip0 = sb.tile([P, NT, 18], F32, tag="ip0")
        d0 = sb.tile([P, NT, 18], F32, tag="d0")
        msk = sb.tile([P, NT, 18], F32, tag="msk")
        ipart = sb.tile([P, NT, 18], F32, tag="ipart")
        frac = sb.tile([P, NT, 18], F32, tag="frac")
        omf = sb.tile([P, NT, 18], F32, tag="omf")
        w4 = sb.tile([P, NT, KK, 4], F32, tag="w4")
        idxf = sb.tile([P, NT, KK], F32, tag="idxf")
        idxs16 = []

        def ysl(t, sl):  # [128, n, 9] strided views (d = 2kk + {0:y, 1:x})
            v = t[:].rearrange("p s (k two) -> p s k two", two=2)
            return v[:, sl, :, 0]

        def xsl(t, sl):
            v = t[:].rearrange("p s (k two) -> p s k two", two=2)
            return v[:, sl, :, 1]

        xpad_src = bass.AP(xpad2, 0, [[2 * C, TBL2 - 1], [1, 4 * C]])
        # gather tiles for (h0, g4=0) taps 0..5: allocated upfront so block
        # 0's subtile can be gathered while the rest of setup still runs.
        # Capped below the gpool depth: a pre-gather whose tile needs a
        # recycled buffer would head-of-line-block the in-order Pool queue.
        NPRE = 6
        gd_pre = [
            gpool.tile([P, 4, 1024], F16, tag="gd", name=f"gd_0_0_{kk}")
            for kk in range(NPRE)
        ]

        setup_ctx = ExitStack()
        setup_ctx.enter_context(tc.high_priority(offset=100000))
        last_setup = {}
        last_bt = []
        # block 0 is a single 128-sample subtile: its indices are ready ~10us
        # before a full quarter's would be, so the first gathers start early
        blocks = [(0, 1), (1, 3), (4, 4), (8, 4), (12, 4)]
        for bi, (s0, n) in enumerate(blocks):
            h = s0 // 8
            sl = slice(s0, s0 + n)
            # stage A: offset conv for this block's rows (n*128 samples)
            ps = psA.tile([P, 512], F32, tag="psoff")
            for t in range(18):
                kk, ch = t // 2, t % 2
                ky, kx = kk // 3, kk % 3
                rhs = t_xchw[:, ch, :].rearrange("p (r w) -> p r w", w=WP)[
                    :, 2 * s0 + ky : 2 * s0 + ky + 2 * n, kx + 3 : kx + 3 + W
                ]
                i_cv = nc.tensor.matmul(
                    ps[0:18, : n * P],
                    lhsT=t_owre[:, t, :],
                    rhs=rhs,
                    start=(t == 0),
                    stop=(t == 17),
                )
                # keep PE stream in setup order: the next conv must not jump
                # ahead of this block's B-transposes (greedy scheduler would)
                if t == 0 and bi > 0:
                    add_dep_helper(i_cv.ins, last_bt[bi - 1].ins,
                                   reason="B-transposes before next conv")
            i_act = nc.scalar.add(
                off_sb[0:18, s0 * P : (s0 + n) * P], ps[0:18, : n * P], t_obcol[0:18, 0:1]
            )
            # stage B: transpose to offT [128, st, 18]; one merged copy
            pst4 = psA.tile([P, 4, 18], F32, tag="pstr", name=f"pst4_{bi}")
            for st in range(s0, s0 + n):
                i_bt = nc.tensor.transpose(
                    pst4[:, st - s0, 0:18],
                    in_=off_sb[0:18, st * P : (st + 1) * P],
                    identity=t_idn18,
                )
            nc.vector.tensor_copy(offT[:, sl, :], pst4[:, 0:n, :])
            last_bt.append(i_bt)
            # stage C: bilinear math on this block's slice [128, n*18]
            bsl = t_base.rearrange("p (s d) -> p s d", d=18)[:, sl, :]
            nc.vector.tensor_add(pP4[:, sl, :], offT[:, sl, :], bsl)
            nc.vector.tensor_scalar(pc[:, sl, :], pP4[:, sl, :], 0.0, 70.999, op0=AL.max, op1=AL.min)
            # floor robust to the f32->i32 cast mode: cast(pc - 0.5) is floor
            # under RNE (hw) but floor-1 for frac<0.5 under truncation
            # (interp); fix with d0 = pc - cast, msk = (d0 >= 1).
            nc.vector.tensor_scalar_add(i32[:, sl, :], pc[:, sl, :], -0.5)
            nc.vector.tensor_sub(d0[:, sl, :], pc[:, sl, :], i32[:, sl, :])
            nc.vector.tensor_scalar(msk[:, sl, :], d0[:, sl, :], 1.0, None, op0=AL.is_ge)
            nc.vector.tensor_add(ipart[:, sl, :], i32[:, sl, :], msk[:, sl, :])
            nc.vector.tensor_sub(frac[:, sl, :], d0[:, sl, :], msk[:, sl, :])
            nc.vector.tensor_scalar(omf[:, sl, :], frac[:, sl, :], -1.0, 1.0, op0=AL.mult, op1=AL.add)
            # w4 corner order of the pair-table fetch:
            # q0=(y0,x0), q1=(y1,x0), q2=(y0,x1), q3=(y1,x1)
            nc.vector.tensor_mul(w4[:, sl, :, 0], ysl(omf, sl), xsl(omf, sl))
            nc.vector.tensor_mul(w4[:, sl, :, 1], ysl(frac, sl), xsl(omf, sl))
            nc.vector.tensor_mul(w4[:, sl, :, 2], ysl(omf, sl), xsl(frac, sl))
            nc.vector.tensor_mul(w4[:, sl, :, 3], ysl(frac, sl), xsl(frac, sl))
            # idxf [128, n, 9]: pair-table row = 72*y0 + x0 (padded coords)
            nc.vector.scalar_tensor_tensor(
                idxf[:, sl, :], ysl(ipart, sl), 72.0, xsl(ipart, sl),
                op0=AL.mult, op1=AL.add,
            )

            # stage D: wrapped idx layout on-chip. Gather call (h, kk, g4)
            # slot i = st8*128 + p needs its idx at wrapped (r, s) = (i%16,
            # i//16) = (p%16, st8*8 + p//16), replicated over 16-partition
            # groups. One matmul per a with the constant selector R_a[p, m] =
            # 1 iff p == a*16 + m%16 yields psum[m, (st8 kk)] = idxf[a*16 +
            # m%16, (st8 kk)] - the wrapped layout, already replicated.
            if s0 % 8 == 0:
                # distinct tags: with a shared tag in this bufs=1 pool the
                # h=1 tile would REUSE h=0's buffer, making its writer wait
                # (WAR) on every h=0 gather — stalling the h transition and
                # poisoning the freed setup-PSUM banks' anti-deps
                idxs16.append(
                    sb.tile([P, KK, 8, 8], I16, tag="idxs16", name=f"idxs16_{h}")
                )
            ih = idxs16[h]
            sth = s0 - 8 * h
            psT8 = psT.tile([P, 8, 4 * KK], F32, tag="psT2", name=f"psT8_{bi}")
            for a in range(8):
                i_pe = nc.tensor.matmul(
                    psT8[:, a, : n * KK],
                    lhsT=t_rsel[:, a, :],
                    rhs=idxf[:, sl, :].rearrange("p a b -> p (a b)"),
                    start=True,
                    stop=True,
                )
            # one strided copy assembles the whole block's wrapped indices.
            # On Act, not DVE: the DVE queue gets stuffed with stage-E diag
            # builds, which would strand this copy (and the first h=1
            # gather behind it) tens of us out.
            i_dve = nc.scalar.copy(
                ih[:, :, sth : sth + n, :].transpose([0, 3, 2, 1]),
                psT8[:, :, : n * KK].rearrange("p a (s k) -> p a s k", k=KK),
            )
            last_setup = {"pe": i_pe, "dve": i_dve, "act": i_act}
            if bi == 0:
                # early gathers: block 0's 128 samples for taps 0..NPRE-1
                for kk in range(NPRE):
                    nc.gpsimd.dma_gather(
                        gd_pre[kk][:, 0:1, :],
                        xpad_src,
                        idxs16[0][:, kk, 0:1, :],
                        num_idxs=128,
                        num_idxs_reg=128,
                        elem_size=4 * C,
                        elem_step=2 * C,
                    )

        setup_ctx.close()
        tc.cur_priority += 500000  # push stage E far behind setup in the ready heap
        if debug_outputs:
            nc.sync.dma_start(out=dbg_off.ap(), in_=off_sb[0:18, :])
            nc.sync.dma_start(out=dbg_w4.ap(), in_=w4[:].rearrange("p a b c -> p (a b c)"))
            for h in range(2):
                nc.sync.dma_start(
                    out=dbg_idx.ap().rearrange("p (h n) -> p h n", h=2)[:, h, :],
                    in_=idxs16[h][:].rearrange("p a b c -> p (a b c)"),
                )

        psAB.close()  # free setup PSUM banks
        psE = es.enter_context(tc.tile_pool(name="psE", bufs=2, space="PSUM"))
        psG = es.enter_context(tc.tile_pool(name="psG", bufs=2, space="PSUM"))

        # ---- stage E: gather + scale + PSUM-accumulate transpose + GEMM.
        # Quarter-major order (h, g4, kk). Each quarter's GEMM is DEFERRED
        # into the next quarter's span (psG bufs=2 keeps both pso sets
        # alive): the deferred flushes read quarter-old sampled data, so
        # they never stall the in-order PE queue and instead fill every PE
        # idle slot. The last quarter flushes inline so nothing is left for
        # the drain but the final taps.
        def mk_flush(pso_, sampled_):
            def flush_gemm(dk):
                for ch in range(2):
                    t = dk * 2 + ch
                    for oh in range(2):
                        nc.tensor.matmul(
                            pso_[oh][:],
                            lhsT=t_wre[:, t, oh * P : (oh + 1) * P],
                            rhs=sampled_[:, t, :],
                            start=(t == 0),
                            stop=(t == 17),
                        )
            return flush_gemm

        def emit_out(pso_, h_, g4_, last_q_):
            for oh in range(2):
                ot = otp.tile([P, 512], F16, tag="ot", name=f"ot_{h_}_{g4_}_{oh}")
                # bias added at the PSUM read (per-partition col); the final
                # quarter's pair splits across Act/DVE so the copies overlap
                # in the drain
                if last_q_ and oh == 1:
                    nc.vector.tensor_scalar_add(ot[:], pso_[oh][:], t_bcols[:, 1:2])
                else:
                    nc.scalar.activation(
                        ot[:],
                        pso_[oh][:],
                        mybir.ActivationFunctionType.Identity,
                        bias=t_bcols[:, oh : oh + 1],
                    )
                nc.sync.dma_start(
                    out=bass.AP(
                        out, oh * P * NS + h_ * 1024 + g4_ * 512, [[NS, P], [1, 512]]
                    ),
                    in_=ot[:],
                )

        prev = None  # (pend, flush_fn, pso, h, g4) of the previous quarter
        for h in range(2):
            for g4 in range(2):
                sampled = smpp.tile(
                    [P, 18, 512], F16, tag="sampled", name=f"smp_{h}_{g4}"
                )
                pso = [
                    psG.tile([P, 512], F32, tag=f"pso{oh}", name=f"pso{oh}_{h}_{g4}")
                    for oh in range(2)
                ]
                pend = []
                flush_gemm = mk_flush(pso, sampled)

                last_q = h == 1 and g4 == 1
                first_q = h == 0 and g4 == 0
                for kk in range(KK):
                    if first_q and kk < NPRE:
                        # subtile 0 was gathered during setup; fetch st 1..3
                        gd = gd_pre[kk]
                        nc.gpsimd.dma_gather(
                            gd[:, 1:4, :],
                            xpad_src,
                            idxs16[h][:, kk, 1:4, :],
                            num_idxs=384,
                            num_idxs_reg=384,
                            elem_size=4 * C,
                            elem_step=2 * C,
                        )
                    elif last_q and kk >= KK - 5:
                        # split the final gathers so the per-subtile combines
                        # overlap the remaining sub-transfers (shrinks the
                        # drain tail)
                        gd = gpool.tile([P, 4, 1024], F16, tag="gd", name=f"gd_{h}_{g4}_{kk}")
                        for i4g in range(4):
                            nc.gpsimd.dma_gather(
                                gd[:, i4g : i4g + 1, :],
                                xpad_src,
                                idxs16[h][:, kk, g4 * 4 + i4g : g4 * 4 + i4g + 1, :],
                                num_idxs=128,
                                num_idxs_reg=128,
                                elem_size=4 * C,
                                elem_step=2 * C,
                            )
                    else:
                        gd = gpool.tile([P, 4, 1024], F16, tag="gd", name=f"gd_{h}_{g4}_{kk}")
                        nc.gpsimd.dma_gather(
                            gd[:],
                            xpad_src,
                            idxs16[h][:, kk, g4 * 4 : (g4 + 1) * 4, :],
                            num_idxs=512,
                            num_idxs_reg=512,
                            elem_size=4 * C,
                            elem_step=2 * C,
                        )
                    ptile = [
                        psE.tile(
                            [P, 512], F32, tag=f"pt{ch}", name=f"pt{ch}_{h}_{g4}_{kk}"
                        )
                        for ch in range(2)
                    ]
                    for i4 in range(4):
                        st8 = g4 * 4 + i4
                        st = h * 8 + st8
                        # diag trick: dg4[:, q, :] = idn * w_q (per-sample
                        # diagonal); PE computes gd_q^T @ diag(w_q), i.e. the
                        # scaled transpose, with the scaling folded into the
                        # matmul rhs. Half the DVE elements of scaling the
                        # [128, 256] corners directly.
                        dg4 = sclp.tile([P, 4, P], F16, tag="dg4")
                        for q in range(4):
                            nc.vector.tensor_scalar_mul(
                                dg4[:, q, :],
                                t_idn16[:],
                                w4[:, st, kk, q : q + 1],
                            )
                        for ch in range(2):
                            for q in range(4):
                                nc.tensor.matmul(
                                    ptile[ch][:, i4 * P : (i4 + 1) * P],
                                    lhsT=gd[:, i4, q * C + ch * P : q * C + (ch + 1) * P],
                                    rhs=dg4[:, q, :],
                                    start=(q == 0),
                                    stop=(q == 3),
                                )
                    for ch in range(2):
                        t = kk * 2 + ch
                        # drain phase of the last quarter: split the copies
                        # across Act/DVE so the PSUM->sampled stage doesn't
                        # serialize the in-order queues
                        if last_q and kk >= 4 and ch == 1:
                            nc.vector.tensor_copy(sampled[:, t, :], ptile[ch][:])
                        else:
                            nc.scalar.copy(sampled[:, t, :], ptile[ch][:])
                    pend.append(kk)
                    if last_q:
                        # drain the leftover previous-quarter flushes, then
                        # stream this quarter's inline with delay 2 (1 for
                        # the final taps)
                        if prev is not None and prev[0]:
                            prev[1](prev[0].pop(0))
                            if kk >= 5 and prev[0]:
                                prev[1](prev[0].pop(0))
                        depth = 1 if kk >= KK - 2 else 2
                        while len(pend) > depth:
                            flush_gemm(pend.pop(0))
                    elif prev is not None and prev[0]:
                        # deferred GEMM of the previous quarter: one K-tile
                        # pair per tap; its data is a whole quarter old, so
                        # these never stall the PE queue — they fill its
                        # idle slots
                        prev[1](prev[0].pop(0))
                if prev is not None:
                    while prev[0]:
                        prev[1](prev[0].pop(0))
                    emit_out(prev[2], prev[3], prev[4], False)
                if last_q:
                    for dk in pend:
                        flush_gemm(dk)
                    emit_out(pso, h, g4, True)
                prev = (pend, flush_gemm, pso, h, g4)
                if debug_outputs:
                    nc.sync.dma_start(
                        out=dbg_smp.ap().rearrange(
                            "p (t q n) -> p t q n", q=4, n=512
                        )[:, :, h * 2 + g4, :],
                        in_=sampled[:],
                    )

    nc.compile()
    return nc


def host_prep(x, weight, bias, offset_w, offset_b):
    """Returns (in_maps list of 8 dicts, assemble fn)."""
    B = x.shape[0]
    xp = np.zeros((B, WP, WP, C), np.float16)
    xp[:, PAD : PAD + H, PAD : PAD + W, :] = x.transpose(0, 2, 3, 1)
    # pair table: row r = [pixel r | pixel r+72] so one 2KB fetch at rows
    # (r, r+1) yields all 4 bilinear corners.
    xpad2_b = []
    for b in range(B):
        flat = xp[b].reshape(NPIX, C)
        t2 = np.zeros((TBL2, 2 * C), np.float16)
        t2[: TBL2 - 1, 0:C] = flat[: TBL2 - 1]
        t2[: TBL2 - 1, C : 2 * C] = flat[72 : TBL2 - 1 + 72]
        xpad2_b.append(t2)
    # c-major padded image for the offset conv, per (b, hh): rows 32h+3 .. +37
    xcp = xp.transpose(0, 3, 1, 2).reshape(B, 2, P, WP, WP)  # [b, grp, 128, 72, 72]
    wre = np.ascontiguousarray(
        weight.reshape(C, 2, P, 3, 3).transpose(3, 4, 1, 2, 0).reshape(KK * 2, P, C)
    ).astype(np.float16)
    # t = kk*2 + ch ; value = offset_w[o, ch*128+i, ky, kx]; packed [P, 18*18]
    owre = np.ascontiguousarray(
        offset_w.reshape(18, 2, P, 3, 3).transpose(2, 3, 4, 1, 0).reshape(P, 18 * 18)
    ).astype(np.float16)
    idn16 = np.eye(P, dtype=np.float16)
    obcol = np.zeros((P, 1), np.float32)
    obcol[:18, 0] = offset_b
    # selector for the wrapped-idx matmuls: rsel[p, a, m] = 1 iff p == a*16 + m%16
    rsel = np.zeros((P, 8, P), np.float32)
    for a in range(8):
        for m in range(P):
            rsel[a * 16 + m % 16, a, m] = 1.0
    rsel = rsel.reshape(P, 8 * P)
    bcols = np.asarray(bias, np.float32).reshape(2, P).T.copy()  # [128, 2]

    base_all = []
    for hh in range(2):
        base = np.zeros((P, NT, 18), np.float32)
        p = np.arange(P)
        for st in range(NT):
            n = st * P + p
            ho = 32 * hh + n // W
            wo = n % W
            for kk in range(KK):
                ky, kx = kk // 3, kk % 3
                base[:, st, 2 * kk + 0] = ky + ho - 1 + PAD
                base[:, st, 2 * kk + 1] = kx + wo - 1 + PAD
        base_all.append(base.reshape(P, NT * 18))

    # packed f32 constants: [obcol | bcols | basep4 | rsel | idn18-rows]
    cpk_all = []
    for hh in range(2):
        cpk = np.zeros((P, 1 + 2 + NT * 18 + 8 * P + 18), np.float32)
        cpk[:, 0:1] = obcol
        cpk[:, 1:3] = bcols
        cpk[:, 3 : 3 + NT * 18] = base_all[hh]
        cpk[:, 3 + NT * 18 : 3 + NT * 18 + 8 * P] = rsel
        cpk[0:18, 3 + NT * 18 + 8 * P :] = np.eye(18, dtype=np.float32)
        cpk_all.append(cpk)

    in_maps = []
    for core in range(8):
        b, hh = core // 2, core % 2
        in_maps.append(
            {
                "xpad2": xpad2_b[b],
                "xchw": np.ascontiguousarray(
                    xcp[b, :, :, 32 * hh + 3 : 32 * hh + 37, :].reshape(2, P, 34 * WP)
                ),
                "wre": wre,
                "owre": owre,
                "cpk": cpk_all[hh],
                "idn16": idn16,
            }
        )

    def assemble(results):
        y = np.empty((B, C, H, W), np.float32)
        for core in range(8):
            b, hh = core // 2, core % 2
            y[b, :, 32 * hh : 32 * (hh + 1), :] = (
                results[core]["out"].astype(np.float32).reshape(C, HO, W)
            )
        return y

    return in_maps, assemble


_CACHE = {}


def _maybe_reset_devices():
    # Clear any wedged accelerator state left by a previous crashed run.
    try:
        import ctypes
        import jax

        jax.devices()
        lib = ctypes.CDLL("/opt/axon/libaxon_pjrt.so")
        if hasattr(lib, "axon_reset"):
            lib.axon_reset.restype = ctypes.c_int64
            lib.axon_reset()
    except Exception:
        pass


def kernel(x, weight, bias, offset_w, offset_b, trace=False):
    if "nc" not in _CACHE:
        _maybe_reset_devices()
        _CACHE["nc"] = build()
    nc = _CACHE["nc"]
    in_maps, assemble = host_prep(
        np.asarray(x), np.asarray(weight), np.asarray(bias),
        np.asarray(offset_w), np.asarray(offset_b),
    )
    res = bass_utils.run_bass_kernel_spmd(
        nc, in_maps, core_ids=list(range(8)), trace=trace
    )
    out = assemble(res.results)
    _CACHE["last_exec_time_ns"] = res.exec_time_ns
    return out



# revision 63
# speedup vs baseline: 1.0057x; 1.0046x over previous
"""DeformConv2d TRN2 kernel: build + host prep + SPMD runner.

Layout/algorithm summary (per core; 8 cores = 4 batches x 2 row-halves):
  - setup runs per row-block (a 1-subtile micro-block first, then 3/4/4/4):
    offset conv (3x3, 18 out ch) as 18 K-tile matmuls -> offsets ->
    PE-transpose to [n-part, 18] -> bilinear ints/fracs/weights on DVE.
    The micro-block makes the first subtile's indices ready ~12us in, and
    its 9 taps are gathered while the rest of setup still runs.
  - gather indices are built ON-CHIP in the wrapped [16-partition, s] layout
    dma_gather requires: one matmul per a with the constant selector
    R_a[p, m] = 1 iff p == a*16 + m%16 maps idxf [n-part, (st8 kk)] to
    psum[m, (st8 kk)] - the wrapped layout already replicated across all
    128 partitions; strided Act copies (f32->i16) assemble idxs16.
  - dma_gather on the PAIR table xpad2[r] = [pix r | pix r+72] with
    elem_size=1024, elem_step=512: ONE descriptor per (sample, tap) fetches
    all 4 bilinear corners (2KB).
  - combine: DVE builds 4 tiny diagonal matrices dg4[:, q, :] = idn * w_q
    (per-sample bilinear weight on the diagonal, [128,128] each); PE then
    computes gd_q^T @ diag(w_q) directly - the scaled transpose with the
    scaling folded into the matmul rhs (half the DVE elements of scaling
    the [128, 256] corners). The 4 corners accumulate in PSUM; Act copies
    PSUM -> sampled [ch, n] f16.
  - stage E runs quarter-major (h, g4, kk). Each quarter's 18-K-tile GEMM
    is mostly deferred into the next quarter's span (psG bufs=2 keeps two
    pso accumulator pairs alive): the deferred K-tiles read quarter-old
    sampled data, so they never stall the in-order PE queue and act as
    filler for its idle slots. The last quarter flushes inline (delay 2,
    then 1) and splits its PSUM->SBUF copies across Act/DVE so the drain
    after the final gather stays short. Bias is added at the PSUM read
    (per-partition bias column on Act / tensor_scalar_add on DVE).

Zero-padding of the table by 4 rows/cols emulates the reference's
valid-masking exactly for |excursion| <= 4; p is clamped to [0, 70.999] in
padded coords so larger offsets also read only zero-pad rows (-> exact 0).
"""

import sys

sys.path.insert(0, "/opt/trn_rl_repo")

import numpy as np

import bass_rust
import concourse.bass as bass
import concourse.bacc as bacc
import concourse.mybir as mybir
import concourse.tile as tile
from concourse import bass_utils
from concourse.tile_rust import add_dep_helper

P = 128
KK = 9
C = 256
H = W = 64
HO = 32          # rows per core (half image)
NS = HO * W      # samples per core = 2048
NT = NS // P     # 16 subtiles of 128 samples
PAD = 4
WP = 72          # padded width/height
NPIX = WP * WP   # 5184 pixels
TBL2 = 5113      # pair-table rows (idx <= 5110, fetch spans rows idx..idx+1)
F16 = mybir.dt.float16
F32 = mybir.dt.float32
I16 = mybir.dt.int16


def build(debug_outputs=False):
    nc = bacc.Bacc("TRN2", num_devices=8, debug=False)

    xpad2 = nc.dram_tensor("xpad2", [TBL2, 2 * C], F16, kind="ExternalInput")
    xchw = nc.dram_tensor("xchw", [2, P, 34 * WP], F16, kind="ExternalInput")
    wre = nc.dram_tensor("wre", [18, P, C], F16, kind="ExternalInput")
    owre = nc.dram_tensor("owre", [P, 18 * 18], F16, kind="ExternalInput")
    basep4 = nc.dram_tensor("basep4", [P, NT * 18], F32, kind="ExternalInput")
    idn16d = nc.dram_tensor("idn16", [P, P], F16, kind="ExternalInput")
    idn18d = nc.dram_tensor("idn18", [18, 18], F32, kind="ExternalInput")
    obcold = nc.dram_tensor("obcol", [P, 1], F32, kind="ExternalInput")
    bcolsd = nc.dram_tensor("bcols", [P, 2], F32, kind="ExternalInput")
    rseld = nc.dram_tensor("rsel", [P, 8 * P], F32, kind="ExternalInput")

    out = nc.dram_tensor("out", [C, NS], F16, kind="ExternalOutput")
    if debug_outputs:
        dbg_off = nc.dram_tensor("dbg_off", [18, NS], F32, kind="ExternalOutput")
        dbg_w4 = nc.dram_tensor("dbg_w4", [P, NT * KK * 4], F32, kind="ExternalOutput")
        dbg_idx = nc.dram_tensor("dbg_idx", [P, 2 * KK * 64], I16, kind="ExternalOutput")
        dbg_smp = nc.dram_tensor("dbg_smp", [P, 18 * NS], F16, kind="ExternalOutput")

    from contextlib import ExitStack

    AL = mybir.AluOpType

    with tile.TileContext(nc) as tc, ExitStack() as es:
        cst = es.enter_context(tc.tile_pool(name="cst", bufs=1))
        sb = es.enter_context(tc.tile_pool(name="sb", bufs=1))
        smpp = es.enter_context(tc.tile_pool(name="smp", bufs=3))
        gpool = es.enter_context(tc.tile_pool(name="gp", bufs=8))
        sclp = es.enter_context(tc.tile_pool(name="scl", bufs=4))
        otp = es.enter_context(tc.tile_pool(name="ot", bufs=4))
        psAB = ExitStack()
        psA = psAB.enter_context(tc.tile_pool(name="psA", bufs=2, space="PSUM"))
        psT = psAB.enter_context(tc.tile_pool(name="psT", bufs=2, space="PSUM"))

        # ---- constants, ordered so the offset-conv dependencies land first:
        # block 0 needs only xchw rows 0..4 + owre + obcol, so those lead.
        t_xchw = cst.tile([P, 2, 34 * WP], F16)
        nc.sync.dma_start(
            out=t_xchw[:, :, : 5 * WP],
            in_=xchw.ap().transpose([1, 0, 2])[:, :, : 5 * WP],
        )
        t_owre = cst.tile([P, 18, 18], F16)
        nc.sync.dma_start(out=t_owre[:], in_=owre.ap().rearrange("p (t d) -> p t d", d=18))
        t_obcolT = cst.tile([P, 1], F32)
        nc.sync.dma_start(out=t_obcolT[:], in_=obcold.ap())
        t_obcol = t_obcolT[:, 0:1]
        t_idn18T = cst.tile([18, 18], F32)
        nc.sync.dma_start(out=t_idn18T[:], in_=idn18d.ap())
        t_idn18 = t_idn18T[:]
        t_baseT = cst.tile([P, NT * 18], F32)
        nc.sync.dma_start(out=t_baseT[:], in_=basep4.ap())
        t_base = t_baseT[:]
        nc.sync.dma_start(
            out=t_xchw[:, :, 5 * WP : 12 * WP],
            in_=xchw.ap().transpose([1, 0, 2])[:, :, 5 * WP : 12 * WP],
        )
        t_rselT = cst.tile([P, 8, P], F32)
        nc.sync.dma_start(out=t_rselT[:], in_=rseld.ap().rearrange("p (a m) -> p a m", m=P))
        t_rsel = t_rselT[:]
        t_idn16 = cst.tile([P, P], F16)
        nc.sync.dma_start(out=t_idn16[:], in_=idn16d.ap())
        nc.sync.dma_start(
            out=t_xchw[:, :, 12 * WP : 23 * WP],
            in_=xchw.ap().transpose([1, 0, 2])[:, :, 12 * WP : 23 * WP],
        )
        nc.sync.dma_start(
            out=t_xchw[:, :, 23 * WP :],
            in_=xchw.ap().transpose([1, 0, 2])[:, :, 23 * WP :],
        )

        t_bcolsT = cst.tile([P, 2], F32)
        nc.sync.dma_start(out=t_bcolsT[:], in_=bcolsd.ap())
        t_bcols = t_bcolsT[:]

        # main-conv weights: needed only once the first gather lands
        t_wre = cst.tile([P, 18, C], F16)
        nc.sync.dma_start(out=t_wre[:], in_=wre.ap().transpose([1, 0, 2]))

        # PE p-state warmup with no DMA dependency (memset source): keeps PE
        # busy from ~t=0.3us so the first conv runs at mid/full clock.
        t_wu = cst.tile([64, 64], F16)
        nc.vector.memset(t_wu[:], 0.5)
        psW = psT.tile([P, P], F32, tag="psW")
        for i in range(40):
            nc.tensor.matmul(
                psW[0:64, 0:64], lhsT=t_wu[:], rhs=t_wu[:],
                start=(i == 0), stop=(i == 39),
            )

        # ---- per-group setup: offset conv -> transpose -> bilinear -> idx
        off_sb = sb.tile([P, NS], F32, tag="offsb")
        offT = sb.tile([P, NT, 18], F32, tag="offT")
        pP4 = sb.tile([P, NT, 18], F32, tag="pP4")
        pc = sb.tile([P, NT, 18], F32, tag="pc")
        i32 = sb.tile([P, NT, 18], mybir.dt.int32, tag="i32")
        ip0 = sb.tile([P, NT, 18], F32, tag="ip0")
        d0 = sb.tile([P, NT, 18], F32, tag="d0")
        msk = sb.tile([P, NT, 18], F32, tag="msk")
        ipart = sb.tile([P, NT, 18], F32, tag="ipart")
        frac = sb.tile([P, NT, 18], F32, tag="frac")
        omf = sb.tile([P, NT, 18], F32, tag="omf")
        w4 = sb.tile([P, NT, KK, 4], F32, tag="w4")
        idxf = sb.tile([P, NT, KK], F32, tag="idxf")
        idxs16 = []

        def ysl(t, sl):  # [128, n, 9] strided views (d = 2kk + {0:y, 1:x})
            v = t[:].rearrange("p s (k two) -> p s k two", two=2)
            return v[:, sl, :, 0]

        def xsl(t, sl):
            v = t[:].rearrange("p s (k two) -> p s k two", two=2)
            return v[:, sl, :, 1]

        xpad_src = bass.AP(xpad2, 0, [[2 * C, TBL2 - 1], [1, 4 * C]])
        # gather tiles for (h0, g4=0) taps 0..5: allocated upfront so block
        # 0's subtile can be gathered while the rest of setup still runs.
        # Capped below the gpool depth: a pre-gather whose tile needs a
        # recycled buffer would head-of-line-block the in-order Pool queue.
        NPRE = 6
        gd_pre = [
            gpool.tile([P, 4, 1024], F16, tag="gd", name=f"gd_0_0_{kk}")
            for kk in range(NPRE)
        ]

        setup_ctx = ExitStack()
        setup_ctx.enter_context(tc.high_priority(offset=100000))
        last_setup = {}
        last_bt = []
        # block 0 is a single 128-sample subtile: its indices are ready ~10us
        # before a full quarter's would be, so the first gathers start early
        blocks = [(0, 1), (1, 3), (4, 4), (8, 4), (12, 4)]
        for bi, (s0, n) in enumerate(blocks):
            h = s0 // 8
            sl = slice(s0, s0 + n)
            # stage A: offset conv for this block's rows (n*128 samples)
            ps = psA.tile([P, 512], F32, tag="psoff")
            for t in range(18):
                kk, ch = t // 2, t % 2
                ky, kx = kk // 3, kk % 3
                rhs = t_xchw[:, ch, :].rearrange("p (r w) -> p r w", w=WP)[
                    :, 2 * s0 + ky : 2 * s0 + ky + 2 * n, kx + 3 : kx + 3 + W
                ]
                i_cv = nc.tensor.matmul(
                    ps[0:18, : n * P],
                    lhsT=t_owre[:, t, :],
                    rhs=rhs,
                    start=(t == 0),
                    stop=(t == 17),
                )
                # keep PE stream in setup order: the next conv must not jump
                # ahead of this block's B-transposes (greedy scheduler would)
                if t == 0 and bi > 0:
                    add_dep_helper(i_cv.ins, last_bt[bi - 1].ins,
                                   reason="B-transposes before next conv")
            i_act = nc.scalar.add(
                off_sb[0:18, s0 * P : (s0 + n) * P], ps[0:18, : n * P], t_obcol[0:18, 0:1]
            )
            # stage B: transpose to offT [128, st, 18]; one merged copy
            pst4 = psA.tile([P, 4, 18], F32, tag="pstr", name=f"pst4_{bi}")
            for st in range(s0, s0 + n):
                i_bt = nc.tensor.transpose(
                    pst4[:, st - s0, 0:18],
                    in_=off_sb[0:18, st * P : (st + 1) * P],
                    identity=t_idn18,
                )
            nc.vector.tensor_copy(offT[:, sl, :], pst4[:, 0:n, :])
            last_bt.append(i_bt)
            # stage C: bilinear math on this block's slice [128, n*18]
            bsl = t_base.rearrange("p (s d) -> p s d", d=18)[:, sl, :]
            nc.vector.tensor_add(pP4[:, sl, :], offT[:, sl, :], bsl)
            nc.vector.tensor_scalar(pc[:, sl, :], pP4[:, sl, :], 0.0, 70.999, op0=AL.max, op1=AL.min)
            # floor robust to the f32->i32 cast mode: cast(pc - 0.5) is floor
            # under RNE (hw) but floor-1 for frac<0.5 under truncation
            # (interp); fix with d0 = pc - cast, msk = (d0 >= 1).
            nc.vector.tensor_scalar_add(i32[:, sl, :], pc[:, sl, :], -0.5)
            nc.vector.tensor_sub(d0[:, sl, :], pc[:, sl, :], i32[:, sl, :])
            nc.vector.tensor_scalar(msk[:, sl, :], d0[:, sl, :], 1.0, None, op0=AL.is_ge)
            nc.vector.tensor_add(ipart[:, sl, :], i32[:, sl, :], msk[:, sl, :])
            nc.vector.tensor_sub(frac[:, sl, :], d0[:, sl, :], msk[:, sl, :])
            nc.vector.tensor_scalar(omf[:, sl, :], frac[:, sl, :], -1.0, 1.0, op0=AL.mult, op1=AL.add)
            # w4 corner order of the pair-table fetch:
            # q0=(y0,x0), q1=(y1,x0), q2=(y0,x1), q3=(y1,x1)
            nc.vector.tensor_mul(w4[:, sl, :, 0], ysl(omf, sl), xsl(omf, sl))
            nc.vector.tensor_mul(w4[:, sl, :, 1], ysl(frac, sl), xsl(omf, sl))
            nc.vector.tensor_mul(w4[:, sl, :, 2], ysl(omf, sl), xsl(frac, sl))
            nc.vector.tensor_mul(w4[:, sl, :, 3], ysl(frac, sl), xsl(frac, sl))
            # idxf [128, n, 9]: pair-table row = 72*y0 + x0 (padded coords)
            nc.vector.scalar_tensor_tensor(
                idxf[:, sl, :], ysl(ipart, sl), 72.0, xsl(ipart, sl),
                op0=AL.mult, op1=AL.add,
            )

            # stage D: wrapped idx layout on-chip. Gather call (h, kk, g4)
            # slot i = st8*128 + p needs its idx at wrapped (r, s) = (i%16,
            # i//16) = (p%16, st8*8 + p//16), replicated over 16-partition
            # groups. One matmul per a with the constant selector R_a[p, m] =
            # 1 iff p == a*16 + m%16 yields psum[m, (st8 kk)] = idxf[a*16 +
            # m%16, (st8 kk)] - the wrapped layout, already replicated.
            if s0 % 8 == 0:
                # distinct tags: with a shared tag in this bufs=1 pool the
                # h=1 tile would REUSE h=0's buffer, making its writer wait
                # (WAR) on every h=0 gather — stalling the h transition and
                # poisoning the freed setup-PSUM banks' anti-deps
                idxs16.append(
                    sb.tile([P, KK, 8, 8], I16, tag="idxs16", name=f"idxs16_{h}")
                )
            ih = idxs16[h]
            sth = s0 - 8 * h
            psT8 = psT.tile([P, 8, 4 * KK], F32, tag="psT2", name=f"psT8_{bi}")
            for a in range(8):
                i_pe = nc.tensor.matmul(
                    psT8[:, a, : n * KK],
                    lhsT=t_rsel[:, a, :],
                    rhs=idxf[:, sl, :].rearrange("p a b -> p (a b)"),
                    start=True,
                    stop=True,
                )
            # one strided copy assembles the whole block's wrapped indices.
            # On Act, not DVE: the DVE queue gets stuffed with stage-E diag
            # builds, which would strand this copy (and the first h=1
            # gather behind it) tens of us out.
            i_dve = nc.scalar.copy(
                ih[:, :, sth : sth + n, :].transpose([0, 3, 2, 1]),
                psT8[:, :, : n * KK].rearrange("p a (s k) -> p a s k", k=KK),
            )
            last_setup = {"pe": i_pe, "dve": i_dve, "act": i_act}
            if bi == 0:
                # early gathers: block 0's 128 samples for taps 0..NPRE-1
                for kk in range(NPRE):
                    nc.gpsimd.dma_gather(
                        gd_pre[kk][:, 0:1, :],
                        xpad_src,
                        idxs16[0][:, kk, 0:1, :],
                        num_idxs=128,
                        num_idxs_reg=128,
                        elem_size=4 * C,
                        elem_step=2 * C,
                    )

        setup_ctx.close()
        tc.cur_priority += 500000  # push stage E far behind setup in the ready heap
        if debug_outputs:
            nc.sync.dma_start(out=dbg_off.ap(), in_=off_sb[0:18, :])
            nc.sync.dma_start(out=dbg_w4.ap(), in_=w4[:].rearrange("p a b c -> p (a b c)"))
            for h in range(2):
                nc.sync.dma_start(
                    out=dbg_idx.ap().rearrange("p (h n) -> p h n", h=2)[:, h, :],
                    in_=idxs16[h][:].rearrange("p a b c -> p (a b c)"),
                )

        psAB.close()  # free setup PSUM banks
        psE = es.enter_context(tc.tile_pool(name="psE", bufs=2, space="PSUM"))
        psG = es.enter_context(tc.tile_pool(name="psG", bufs=2, space="PSUM"))

        # ---- stage E: gather + scale + PSUM-accumulate transpose + GEMM.
        # Quarter-major order (h, g4, kk). Each quarter's GEMM is DEFERRED
        # into the next quarter's span (psG bufs=2 keeps both pso sets
        # alive): the deferred flushes read quarter-old sampled data, so
        # they never stall the in-order PE queue and instead fill every PE
        # idle slot. The last quarter flushes inline so nothing is left for
        # the drain but the final taps.
        def mk_flush(pso_, sampled_):
            def flush_gemm(dk):
                for ch in range(2):
                    t = dk * 2 + ch
                    for oh in range(2):
                        nc.tensor.matmul(
                            pso_[oh][:],
                            lhsT=t_wre[:, t, oh * P : (oh + 1) * P],
                            rhs=sampled_[:, t, :],
                            start=(t == 0),
                            stop=(t == 17),
                        )
            return flush_gemm

        def emit_out(pso_, h_, g4_, last_q_):
            for oh in range(2):
                ot = otp.tile([P, 512], F16, tag="ot", name=f"ot_{h_}_{g4_}_{oh}")
                # bias added at the PSUM read (per-partition col); the final
                # quarter's pair splits across Act/DVE so the copies overlap
                # in the drain
                if last_q_ and oh == 1:
                    nc.vector.tensor_scalar_add(ot[:], pso_[oh][:], t_bcols[:, 1:2])
                else:
                    nc.scalar.activation(
                        ot[:],
                        pso_[oh][:],
                        mybir.ActivationFunctionType.Identity,
                        bias=t_bcols[:, oh : oh + 1],
                    )
                nc.sync.dma_start(
                    out=bass.AP(
                        out, oh * P * NS + h_ * 1024 + g4_ * 512, [[NS, P], [1, 512]]
                    ),
                    in_=ot[:],
                )

        prev = None  # (pend, flush_fn, pso, h, g4) of the previous quarter
        for h in range(2):
            for g4 in range(2):
                sampled = smpp.tile(
                    [P, 18, 512], F16, tag="sampled", name=f"smp_{h}_{g4}"
                )
                pso = [
                    psG.tile([P, 512], F32, tag=f"pso{oh}", name=f"pso{oh}_{h}_{g4}")
                    for oh in range(2)
                ]
                pend = []
                flush_gemm = mk_flush(pso, sampled)

                last_q = h == 1 and g4 == 1
                first_q = h == 0 and g4 == 0
                for kk in range(KK):
                    if first_q and kk < NPRE:
                        # subtile 0 was gathered during setup; fetch st 1..3
                        gd = gd_pre[kk]
                        nc.gpsimd.dma_gather(
                            gd[:, 1:4, :],
                            xpad_src,
                            idxs16[h][:, kk, 1:4, :],
                            num_idxs=384,
                            num_idxs_reg=384,
                            elem_size=4 * C,
                            elem_step=2 * C,
                        )
                    elif last_q and kk >= KK - 5:
                        # split the final gathers so the per-subtile combines
                        # overlap the remaining sub-transfers (shrinks the
                        # drain tail)
                        gd = gpool.tile([P, 4, 1024], F16, tag="gd", name=f"gd_{h}_{g4}_{kk}")
                        for i4g in range(4):
                            nc.gpsimd.dma_gather(
                                gd[:, i4g : i4g + 1, :],
                                xpad_src,
                                idxs16[h][:, kk, g4 * 4 + i4g : g4 * 4 + i4g + 1, :],
                                num_idxs=128,
                                num_idxs_reg=128,
                                elem_size=4 * C,
                                elem_step=2 * C,
                            )
                    else:
                        gd = gpool.tile([P, 4, 1024], F16, tag="gd", name=f"gd_{h}_{g4}_{kk}")
                        nc.gpsimd.dma_gather(
                            gd[:],
                            xpad_src,
                            idxs16[h][:, kk, g4 * 4 : (g4 + 1) * 4, :],
                            num_idxs=512,
                            num_idxs_reg=512,
                            elem_size=4 * C,
                            elem_step=2 * C,
                        )
                    ptile = [
                        psE.tile(
                            [P, 512], F32, tag=f"pt{ch}", name=f"pt{ch}_{h}_{g4}_{kk}"
                        )
                        for ch in range(2)
                    ]
                    for i4 in range(4):
                        st8 = g4 * 4 + i4
                        st = h * 8 + st8
                        # diag trick: dg4[:, q, :] = idn * w_q (per-sample
                        # diagonal); PE computes gd_q^T @ diag(w_q), i.e. the
                        # scaled transpose, with the scaling folded into the
                        # matmul rhs. Half the DVE elements of scaling the
                        # [128, 256] corners directly.
                        dg4 = sclp.tile([P, 4, P], F16, tag="dg4")
                        for q in range(4):
                            nc.vector.tensor_scalar_mul(
                                dg4[:, q, :],
                                t_idn16[:],
                                w4[:, st, kk, q : q + 1],
                            )
                        for ch in range(2):
                            for q in range(4):
                                nc.tensor.matmul(
                                    ptile[ch][:, i4 * P : (i4 + 1) * P],
                                    lhsT=gd[:, i4, q * C + ch * P : q * C + (ch + 1) * P],
                                    rhs=dg4[:, q, :],
                                    start=(q == 0),
                                    stop=(q == 3),
                                )
                    for ch in range(2):
                        t = kk * 2 + ch
                        # drain phase of the last quarter: split the copies
                        # across Act/DVE so the PSUM->sampled stage doesn't
                        # serialize the in-order queues
                        if last_q and kk >= 4 and ch == 1:
                            nc.vector.tensor_copy(sampled[:, t, :], ptile[ch][:])
                        else:
                            nc.scalar.copy(sampled[:, t, :], ptile[ch][:])
                    pend.append(kk)
                    if last_q:
                        # drain the leftover previous-quarter flushes, then
                        # stream this quarter's inline with delay 2 (1 for
                        # the final taps)
                        if prev is not None and prev[0]:
                            prev[1](prev[0].pop(0))
                            if kk >= 5 and prev[0]:
                                prev[1](prev[0].pop(0))
                        depth = 1 if kk >= KK - 2 else 2
                        while len(pend) > depth:
                            flush_gemm(pend.pop(0))
                    elif prev is not None and prev[0]:
                        # deferred GEMM of the previous quarter: one K-tile
                        # pair per tap; its data is a whole quarter old, so
                        # these never stall the PE queue — they fill its
                        # idle slots
                        prev[1](prev[0].pop(0))
                if prev is not None:
                    while prev[0]:
                        prev[1](prev[0].pop(0))
                    emit_out(prev[2], prev[3], prev[4], False)
                if last_q:
                    for dk in pend:
                        flush_gemm(dk)
                    emit_out(pso, h, g4, True)
                prev = (pend, flush_gemm, pso, h, g4)
                if debug_outputs:
                    nc.sync.dma_start(
                        out=dbg_smp.ap().rearrange(
                            "p (t q n) -> p t q n", q=4, n=512
                        )[:, :, h * 2 + g4, :],
                        in_=sampled[:],
                    )

    nc.compile()
    return nc


def host_prep(x, weight, bias, offset_w, offset_b):
    """Returns (in_maps list of 8 dicts, assemble fn)."""
    B = x.shape[0]
    xp = np.zeros((B, WP, WP, C), np.float16)
    xp[:, PAD : PAD + H, PAD : PAD + W, :] = x.transpose(0, 2, 3, 1)
    # pair table: row r = [pixel r | pixel r+72] so one 2KB fetch at rows
    # (r, r+1) yields all 4 bilinear corners.
    xpad2_b = []
    for b in range(B):
        flat = xp[b].reshape(NPIX, C)
        t2 = np.zeros((TBL2, 2 * C), np.float16)
        t2[: TBL2 - 1, 0:C] = flat[: TBL2 - 1]
        t2[: TBL2 - 1, C : 2 * C] = flat[72 : TBL2 - 1 + 72]
        xpad2_b.append(t2)
    # c-major padded image for the offset conv, per (b, hh): rows 32h+3 .. +37
    xcp = xp.transpose(0, 3, 1, 2).reshape(B, 2, P, WP, WP)  # [b, grp, 128, 72, 72]
    wre = np.ascontiguousarray(
        weight.reshape(C, 2, P, 3, 3).transpose(3, 4, 1, 2, 0).reshape(KK * 2, P, C)
    ).astype(np.float16)
    # t = kk*2 + ch ; value = offset_w[o, ch*128+i, ky, kx]; packed [P, 18*18]
    owre = np.ascontiguousarray(
        offset_w.reshape(18, 2, P, 3, 3).transpose(2, 3, 4, 1, 0).reshape(P, 18 * 18)
    ).astype(np.float16)
    idn16 = np.eye(P, dtype=np.float16)
    obcol = np.zeros((P, 1), np.float32)
    obcol[:18, 0] = offset_b
    # selector for the wrapped-idx matmuls: rsel[p, a, m] = 1 iff p == a*16 + m%16
    rsel = np.zeros((P, 8, P), np.float32)
    for a in range(8):
        for m in range(P):
            rsel[a * 16 + m % 16, a, m] = 1.0
    rsel = rsel.reshape(P, 8 * P)
    bcols = np.asarray(bias, np.float32).reshape(2, P).T.copy()  # [128, 2]

    base_all = []
    for hh in range(2):
        base = np.zeros((P, NT, 18), np.float32)
        p = np.arange(P)
        for st in range(NT):
            n = st * P + p
            ho = 32 * hh + n // W
            wo = n % W
            for kk in range(KK):
                ky, kx = kk // 3, kk % 3
                base[:, st, 2 * kk + 0] = ky + ho - 1 + PAD
                base[:, st, 2 * kk + 1] = kx + wo - 1 + PAD
        base_all.append(base.reshape(P, NT * 18))


    in_maps = []
    for core in range(8):
        b, hh = core // 2, core % 2
        in_maps.append(
            {
                "xpad2": xpad2_b[b],
                "xchw": np.ascontiguousarray(
                    xcp[b, :, :, 32 * hh + 3 : 32 * hh + 37, :].reshape(2, P, 34 * WP)
                ),
                "wre": wre,
                "owre": owre,
                "basep4": base_all[hh],
                "idn16": idn16,
                "idn18": np.eye(18, dtype=np.float32),
                "obcol": obcol,
                "bcols": bcols,
                "rsel": rsel,
            }
        )

    def assemble(results):
        y = np.empty((B, C, H, W), np.float32)
        for core in range(8):
            b, hh = core // 2, core % 2
            y[b, :, 32 * hh : 32 * (hh + 1), :] = (
                results[core]["out"].astype(np.float32).reshape(C, HO, W)
            )
        return y

    return in_maps, assemble


_CACHE = {}


def _maybe_reset_devices():
    # Clear any wedged accelerator state left by a previous crashed run.
    try:
        import ctypes
        import jax

        jax.devices()
        lib = ctypes.CDLL("/opt/axon/libaxon_pjrt.so")
        if hasattr(lib, "axon_reset"):
            lib.axon_reset.restype = ctypes.c_int64
            lib.axon_reset()
    except Exception:
        pass


def kernel(x, weight, bias, offset_w, offset_b, trace=False):
    if "nc" not in _CACHE:
        _maybe_reset_devices()
        _CACHE["nc"] = build()
    nc = _CACHE["nc"]
    in_maps, assemble = host_prep(
        np.asarray(x), np.asarray(weight), np.asarray(bias),
        np.asarray(offset_w), np.asarray(offset_b),
    )
    res = bass_utils.run_bass_kernel_spmd(
        nc, in_maps, core_ids=list(range(8)), trace=trace
    )
    out = assemble(res.results)
    _CACHE["last_exec_time_ns"] = res.exec_time_ns
    return out



# revision 64
# speedup vs baseline: 1.0080x; 1.0024x over previous
"""DeformConv2d TRN2 kernel: build + host prep + SPMD runner.

Layout/algorithm summary (per core; 8 cores = 4 batches x 2 row-halves):
  - setup runs per row-block (a 1-subtile micro-block first, then 3/4/4/4):
    offset conv (3x3, 18 out ch) as 18 K-tile matmuls -> offsets ->
    PE-transpose to [n-part, 18] -> bilinear ints/fracs/weights on DVE.
    The micro-block makes the first subtile's indices ready ~12us in, and
    its 9 taps are gathered while the rest of setup still runs.
  - gather indices are built ON-CHIP in the wrapped [16-partition, s] layout
    dma_gather requires: one matmul per a with the constant selector
    R_a[p, m] = 1 iff p == a*16 + m%16 maps idxf [n-part, (st8 kk)] to
    psum[m, (st8 kk)] - the wrapped layout already replicated across all
    128 partitions; strided Act copies (f32->i16) assemble idxs16.
  - dma_gather on the PAIR table xpad2[r] = [pix r | pix r+72] with
    elem_size=1024, elem_step=512: ONE descriptor per (sample, tap) fetches
    all 4 bilinear corners (2KB).
  - combine: DVE builds 4 tiny diagonal matrices dg4[:, q, :] = idn * w_q
    (per-sample bilinear weight on the diagonal, [128,128] each); PE then
    computes gd_q^T @ diag(w_q) directly - the scaled transpose with the
    scaling folded into the matmul rhs (half the DVE elements of scaling
    the [128, 256] corners). The 4 corners accumulate in PSUM; Act copies
    PSUM -> sampled [ch, n] f16.
  - stage E runs quarter-major (h, g4, kk). Each quarter's 18-K-tile GEMM
    is mostly deferred into the next quarter's span (psG bufs=2 keeps two
    pso accumulator pairs alive): the deferred K-tiles read quarter-old
    sampled data, so they never stall the in-order PE queue and act as
    filler for its idle slots. The last quarter flushes inline (delay 2,
    then 1) and splits its PSUM->SBUF copies across Act/DVE so the drain
    after the final gather stays short. Bias is added at the PSUM read
    (per-partition bias column on Act / tensor_scalar_add on DVE).

Zero-padding of the table by 4 rows/cols emulates the reference's
valid-masking exactly for |excursion| <= 4; p is clamped to [0, 70.999] in
padded coords so larger offsets also read only zero-pad rows (-> exact 0).
"""

import sys

sys.path.insert(0, "/opt/trn_rl_repo")

import numpy as np

import bass_rust
import concourse.bass as bass
import concourse.bacc as bacc
import concourse.mybir as mybir
import concourse.tile as tile
from concourse import bass_utils
from concourse.tile_rust import add_dep_helper

P = 128
KK = 9
C = 256
H = W = 64
HO = 32          # rows per core (half image)
NS = HO * W      # samples per core = 2048
NT = NS // P     # 16 subtiles of 128 samples
PAD = 4
WP = 72          # padded width/height
NPIX = WP * WP   # 5184 pixels
TBL2 = 5113      # pair-table rows (idx <= 5110, fetch spans rows idx..idx+1)
F16 = mybir.dt.float16
F32 = mybir.dt.float32
I16 = mybir.dt.int16


def build(debug_outputs=False):
    nc = bacc.Bacc("TRN2", num_devices=8, debug=False)

    xpad2 = nc.dram_tensor("xpad2", [TBL2, 2 * C], F16, kind="ExternalInput")
    xchw = nc.dram_tensor("xchw", [2, P, 34 * WP], F16, kind="ExternalInput")
    wre = nc.dram_tensor("wre", [18, P, C], F16, kind="ExternalInput")
    owre = nc.dram_tensor("owre", [P, 18 * 18], F16, kind="ExternalInput")
    basep4 = nc.dram_tensor("basep4", [P, NT * 18], F32, kind="ExternalInput")
    idn16d = nc.dram_tensor("idn16", [P, P], F16, kind="ExternalInput")
    idn18d = nc.dram_tensor("idn18", [18, 18], F32, kind="ExternalInput")
    obcold = nc.dram_tensor("obcol", [P, 1], F32, kind="ExternalInput")
    bcolsd = nc.dram_tensor("bcols", [P, 2], F32, kind="ExternalInput")
    rseld = nc.dram_tensor("rsel", [P, 8 * P], F32, kind="ExternalInput")

    out = nc.dram_tensor("out", [C, NS], F16, kind="ExternalOutput")
    if debug_outputs:
        dbg_off = nc.dram_tensor("dbg_off", [18, NS], F32, kind="ExternalOutput")
        dbg_w4 = nc.dram_tensor("dbg_w4", [P, NT * KK * 4], F32, kind="ExternalOutput")
        dbg_idx = nc.dram_tensor("dbg_idx", [P, 2 * KK * 64], I16, kind="ExternalOutput")
        dbg_smp = nc.dram_tensor("dbg_smp", [P, 18 * NS], F16, kind="ExternalOutput")

    from contextlib import ExitStack

    AL = mybir.AluOpType

    with tile.TileContext(nc) as tc, ExitStack() as es:
        cst = es.enter_context(tc.tile_pool(name="cst", bufs=1))
        sb = es.enter_context(tc.tile_pool(name="sb", bufs=1))
        smpp = es.enter_context(tc.tile_pool(name="smp", bufs=3))
        gpool = es.enter_context(tc.tile_pool(name="gp", bufs=8))
        sclp = es.enter_context(tc.tile_pool(name="scl", bufs=4))
        otp = es.enter_context(tc.tile_pool(name="ot", bufs=4))
        psAB = ExitStack()
        psA = psAB.enter_context(tc.tile_pool(name="psA", bufs=2, space="PSUM"))
        psT = psAB.enter_context(tc.tile_pool(name="psT", bufs=2, space="PSUM"))

        # ---- constants, ordered so the offset-conv dependencies land first:
        # block 0 needs only xchw rows 0..4 + owre + obcol, so those lead.
        t_xchw = cst.tile([P, 2, 34 * WP], F16)
        nc.sync.dma_start(
            out=t_xchw[:, :, : 5 * WP],
            in_=xchw.ap().transpose([1, 0, 2])[:, :, : 5 * WP],
        )
        t_owre = cst.tile([P, 18, 18], F16)
        nc.sync.dma_start(out=t_owre[:], in_=owre.ap().rearrange("p (t d) -> p t d", d=18))
        t_obcolT = cst.tile([P, 1], F32)
        nc.sync.dma_start(out=t_obcolT[:], in_=obcold.ap())
        t_obcol = t_obcolT[:, 0:1]
        t_idn18T = cst.tile([18, 18], F32)
        nc.sync.dma_start(out=t_idn18T[:], in_=idn18d.ap())
        t_idn18 = t_idn18T[:]
        t_baseT = cst.tile([P, NT * 18], F32)
        nc.sync.dma_start(out=t_baseT[:], in_=basep4.ap())
        t_base = t_baseT[:]
        nc.sync.dma_start(
            out=t_xchw[:, :, 5 * WP : 12 * WP],
            in_=xchw.ap().transpose([1, 0, 2])[:, :, 5 * WP : 12 * WP],
        )
        t_rselT = cst.tile([P, 8, P], F32)
        nc.sync.dma_start(out=t_rselT[:], in_=rseld.ap().rearrange("p (a m) -> p a m", m=P))
        t_rsel = t_rselT[:]
        t_idn16 = cst.tile([P, P], F16)
        nc.sync.dma_start(out=t_idn16[:], in_=idn16d.ap())
        nc.sync.dma_start(
            out=t_xchw[:, :, 12 * WP : 23 * WP],
            in_=xchw.ap().transpose([1, 0, 2])[:, :, 12 * WP : 23 * WP],
        )
        nc.sync.dma_start(
            out=t_xchw[:, :, 23 * WP :],
            in_=xchw.ap().transpose([1, 0, 2])[:, :, 23 * WP :],
        )

        t_bcolsT = cst.tile([P, 2], F32)
        nc.sync.dma_start(out=t_bcolsT[:], in_=bcolsd.ap())
        t_bcols = t_bcolsT[:]

        # main-conv weights: needed only once the first gather lands
        t_wre = cst.tile([P, 18, C], F16)
        nc.sync.dma_start(out=t_wre[:], in_=wre.ap().transpose([1, 0, 2]))

        # PE p-state warmup with no DMA dependency (memset source): keeps PE
        # busy from ~t=0.3us so the first conv runs at mid/full clock.
        t_wu = cst.tile([64, 64], F16)
        nc.vector.memset(t_wu[:], 0.5)
        psW = psT.tile([P, P], F32, tag="psW")
        for i in range(40):
            nc.tensor.matmul(
                psW[0:64, 0:64], lhsT=t_wu[:], rhs=t_wu[:],
                start=(i == 0), stop=(i == 39),
            )

        # ---- per-group setup: offset conv -> transpose -> bilinear -> idx
        off_sb = sb.tile([P, NS], F32, tag="offsb")
        offT = sb.tile([P, NT, 18], F32, tag="offT")
        pP4 = sb.tile([P, NT, 18], F32, tag="pP4")
        pc = sb.tile([P, NT, 18], F32, tag="pc")
        i32 = sb.tile([P, NT, 18], mybir.dt.int32, tag="i32")
        ip0 = sb.tile([P, NT, 18], F32, tag="ip0")
        d0 = sb.tile([P, NT, 18], F32, tag="d0")
        msk = sb.tile([P, NT, 18], F32, tag="msk")
        ipart = sb.tile([P, NT, 18], F32, tag="ipart")
        frac = sb.tile([P, NT, 18], F32, tag="frac")
        omf = sb.tile([P, NT, 18], F32, tag="omf")
        w4 = sb.tile([P, NT, KK, 4], F32, tag="w4")
        idxf = sb.tile([P, NT, KK], F32, tag="idxf")
        idxs16 = []

        def ysl(t, sl):  # [128, n, 9] strided views (d = 2kk + {0:y, 1:x})
            v = t[:].rearrange("p s (k two) -> p s k two", two=2)
            return v[:, sl, :, 0]

        def xsl(t, sl):
            v = t[:].rearrange("p s (k two) -> p s k two", two=2)
            return v[:, sl, :, 1]

        xpad_src = bass.AP(xpad2, 0, [[2 * C, TBL2 - 1], [1, 4 * C]])
        # gather tiles for (h0, g4=0) taps 0..5: allocated upfront so block
        # 0's subtile can be gathered while the rest of setup still runs.
        # Capped below the gpool depth: a pre-gather whose tile needs a
        # recycled buffer would head-of-line-block the in-order Pool queue.
        NPRE = 6
        gd_pre = [
            gpool.tile([P, 4, 1024], F16, tag="gd", name=f"gd_0_0_{kk}")
            for kk in range(NPRE)
        ]

        setup_ctx = ExitStack()
        setup_ctx.enter_context(tc.high_priority(offset=100000))
        last_setup = {}
        last_bt = []
        # block 0 is a single 128-sample subtile: its indices are ready ~10us
        # before a full quarter's would be, so the first gathers start early
        blocks = [(0, 1), (1, 3), (4, 4), (8, 4), (12, 4)]
        for bi, (s0, n) in enumerate(blocks):
            h = s0 // 8
            sl = slice(s0, s0 + n)
            # stage A: offset conv for this block's rows (n*128 samples)
            ps = psA.tile([P, 512], F32, tag="psoff")
            for t in range(18):
                kk, ch = t // 2, t % 2
                ky, kx = kk // 3, kk % 3
                rhs = t_xchw[:, ch, :].rearrange("p (r w) -> p r w", w=WP)[
                    :, 2 * s0 + ky : 2 * s0 + ky + 2 * n, kx + 3 : kx + 3 + W
                ]
                i_cv = nc.tensor.matmul(
                    ps[0:18, : n * P],
                    lhsT=t_owre[:, t, :],
                    rhs=rhs,
                    start=(t == 0),
                    stop=(t == 17),
                )
                # keep PE stream in setup order: the next conv must not jump
                # ahead of this block's B-transposes (greedy scheduler would)
                if t == 0 and bi > 0:
                    add_dep_helper(i_cv.ins, last_bt[bi - 1].ins,
                                   reason="B-transposes before next conv")
            i_act = nc.scalar.add(
                off_sb[0:18, s0 * P : (s0 + n) * P], ps[0:18, : n * P], t_obcol[0:18, 0:1]
            )
            # stage B: transpose to offT [128, st, 18]; one merged copy
            pst4 = psA.tile([P, 4, 18], F32, tag="pstr", name=f"pst4_{bi}")
            for st in range(s0, s0 + n):
                i_bt = nc.tensor.transpose(
                    pst4[:, st - s0, 0:18],
                    in_=off_sb[0:18, st * P : (st + 1) * P],
                    identity=t_idn18,
                )
            nc.vector.tensor_copy(offT[:, sl, :], pst4[:, 0:n, :])
            last_bt.append(i_bt)
            # stage C: bilinear math on this block's slice [128, n*18]
            bsl = t_base.rearrange("p (s d) -> p s d", d=18)[:, sl, :]
            nc.vector.tensor_add(pP4[:, sl, :], offT[:, sl, :], bsl)
            nc.vector.tensor_scalar(pc[:, sl, :], pP4[:, sl, :], 0.0, 70.999, op0=AL.max, op1=AL.min)
            # floor robust to the f32->i32 cast mode: cast(pc - 0.5) is floor
            # under RNE (hw) but floor-1 for frac<0.5 under truncation
            # (interp); fix with d0 = pc - cast, msk = (d0 >= 1).
            nc.vector.tensor_scalar_add(i32[:, sl, :], pc[:, sl, :], -0.5)
            nc.vector.tensor_sub(d0[:, sl, :], pc[:, sl, :], i32[:, sl, :])
            nc.vector.tensor_scalar(msk[:, sl, :], d0[:, sl, :], 1.0, None, op0=AL.is_ge)
            nc.vector.tensor_add(ipart[:, sl, :], i32[:, sl, :], msk[:, sl, :])
            nc.vector.tensor_sub(frac[:, sl, :], d0[:, sl, :], msk[:, sl, :])
            nc.vector.tensor_scalar(omf[:, sl, :], frac[:, sl, :], -1.0, 1.0, op0=AL.mult, op1=AL.add)
            # w4 corner order of the pair-table fetch:
            # q0=(y0,x0), q1=(y1,x0), q2=(y0,x1), q3=(y1,x1)
            nc.vector.tensor_mul(w4[:, sl, :, 0], ysl(omf, sl), xsl(omf, sl))
            nc.vector.tensor_mul(w4[:, sl, :, 1], ysl(frac, sl), xsl(omf, sl))
            nc.vector.tensor_mul(w4[:, sl, :, 2], ysl(omf, sl), xsl(frac, sl))
            nc.vector.tensor_mul(w4[:, sl, :, 3], ysl(frac, sl), xsl(frac, sl))
            # idxf [128, n, 9]: pair-table row = 72*y0 + x0 (padded coords)
            nc.vector.scalar_tensor_tensor(
                idxf[:, sl, :], ysl(ipart, sl), 72.0, xsl(ipart, sl),
                op0=AL.mult, op1=AL.add,
            )

            # stage D: wrapped idx layout on-chip. Gather call (h, kk, g4)
            # slot i = st8*128 + p needs its idx at wrapped (r, s) = (i%16,
            # i//16) = (p%16, st8*8 + p//16), replicated over 16-partition
            # groups. One matmul per a with the constant selector R_a[p, m] =
            # 1 iff p == a*16 + m%16 yields psum[m, (st8 kk)] = idxf[a*16 +
            # m%16, (st8 kk)] - the wrapped layout, already replicated.
            if s0 % 8 == 0:
                # distinct tags: with a shared tag in this bufs=1 pool the
                # h=1 tile would REUSE h=0's buffer, making its writer wait
                # (WAR) on every h=0 gather — stalling the h transition and
                # poisoning the freed setup-PSUM banks' anti-deps
                idxs16.append(
                    sb.tile([P, KK, 8, 8], I16, tag="idxs16", name=f"idxs16_{h}")
                )
            ih = idxs16[h]
            sth = s0 - 8 * h
            psT8 = psT.tile([P, 8, 4 * KK], F32, tag="psT2", name=f"psT8_{bi}")
            for a in range(8):
                i_pe = nc.tensor.matmul(
                    psT8[:, a, : n * KK],
                    lhsT=t_rsel[:, a, :],
                    rhs=idxf[:, sl, :].rearrange("p a b -> p (a b)"),
                    start=True,
                    stop=True,
                )
            # one strided copy assembles the whole block's wrapped indices.
            # On Act, not DVE: the DVE queue gets stuffed with stage-E diag
            # builds, which would strand this copy (and the first h=1
            # gather behind it) tens of us out.
            i_dve = nc.scalar.copy(
                ih[:, :, sth : sth + n, :].transpose([0, 3, 2, 1]),
                psT8[:, :, : n * KK].rearrange("p a (s k) -> p a s k", k=KK),
            )
            last_setup = {"pe": i_pe, "dve": i_dve, "act": i_act}
            if bi == 0:
                # early gathers: block 0's 128 samples for taps 0..NPRE-1
                for kk in range(NPRE):
                    nc.gpsimd.dma_gather(
                        gd_pre[kk][:, 0:1, :],
                        xpad_src,
                        idxs16[0][:, kk, 0:1, :],
                        num_idxs=128,
                        num_idxs_reg=128,
                        elem_size=4 * C,
                        elem_step=2 * C,
                    )

        setup_ctx.close()
        tc.cur_priority += 500000  # push stage E far behind setup in the ready heap
        if debug_outputs:
            nc.sync.dma_start(out=dbg_off.ap(), in_=off_sb[0:18, :])
            nc.sync.dma_start(out=dbg_w4.ap(), in_=w4[:].rearrange("p a b c -> p (a b c)"))
            for h in range(2):
                nc.sync.dma_start(
                    out=dbg_idx.ap().rearrange("p (h n) -> p h n", h=2)[:, h, :],
                    in_=idxs16[h][:].rearrange("p a b c -> p (a b c)"),
                )

        psAB.close()  # free setup PSUM banks
        psE = es.enter_context(tc.tile_pool(name="psE", bufs=2, space="PSUM"))
        psG = es.enter_context(tc.tile_pool(name="psG", bufs=2, space="PSUM"))

        # ---- stage E: gather + scale + PSUM-accumulate transpose + GEMM.
        # Quarter-major order (h, g4, kk). Each quarter's GEMM is DEFERRED
        # into the next quarter's span (psG bufs=2 keeps both pso sets
        # alive): the deferred flushes read quarter-old sampled data, so
        # they never stall the in-order PE queue and instead fill every PE
        # idle slot. The last quarter flushes inline so nothing is left for
        # the drain but the final taps.
        def mk_flush(pso_, sampled_):
            def flush_gemm(dk):
                for ch in range(2):
                    t = dk * 2 + ch
                    for oh in range(2):
                        nc.tensor.matmul(
                            pso_[oh][:],
                            lhsT=t_wre[:, t, oh * P : (oh + 1) * P],
                            rhs=sampled_[:, t, :],
                            start=(t == 0),
                            stop=(t == 17),
                        )
            return flush_gemm

        def emit_out(pso_, h_, g4_, last_q_):
            for oh in range(2):
                ot = otp.tile([P, 512], F16, tag="ot", name=f"ot_{h_}_{g4_}_{oh}")
                # bias added at the PSUM read (per-partition col); the final
                # quarter's pair splits across Act/DVE so the copies overlap
                # in the drain
                if last_q_ and oh == 1:
                    nc.vector.tensor_scalar_add(ot[:], pso_[oh][:], t_bcols[:, 1:2])
                else:
                    nc.scalar.activation(
                        ot[:],
                        pso_[oh][:],
                        mybir.ActivationFunctionType.Identity,
                        bias=t_bcols[:, oh : oh + 1],
                    )
                nc.sync.dma_start(
                    out=bass.AP(
                        out, oh * P * NS + h_ * 1024 + g4_ * 512, [[NS, P], [1, 512]]
                    ),
                    in_=ot[:],
                )

        prev = None  # (pend, flush_fn, pso, h, g4) of the previous quarter
        for h in range(2):
            for g4 in range(2):
                sampled = smpp.tile(
                    [P, 18, 512], F16, tag="sampled", name=f"smp_{h}_{g4}"
                )
                pso = [
                    psG.tile([P, 512], F32, tag=f"pso{oh}", name=f"pso{oh}_{h}_{g4}")
                    for oh in range(2)
                ]
                pend = []
                flush_gemm = mk_flush(pso, sampled)

                last_q = h == 1 and g4 == 1
                first_q = h == 0 and g4 == 0
                for kk in range(KK):
                    if first_q and kk < NPRE:
                        # subtile 0 was gathered during setup; fetch st 1..3
                        gd = gd_pre[kk]
                        nc.gpsimd.dma_gather(
                            gd[:, 1:4, :],
                            xpad_src,
                            idxs16[h][:, kk, 1:4, :],
                            num_idxs=384,
                            num_idxs_reg=384,
                            elem_size=4 * C,
                            elem_step=2 * C,
                        )
                    elif last_q and kk >= KK - 7:
                        # split the final gathers so the per-subtile combines
                        # overlap the remaining sub-transfers (shrinks the
                        # drain tail)
                        gd = gpool.tile([P, 4, 1024], F16, tag="gd", name=f"gd_{h}_{g4}_{kk}")
                        for i4g in range(4):
                            nc.gpsimd.dma_gather(
                                gd[:, i4g : i4g + 1, :],
                                xpad_src,
                                idxs16[h][:, kk, g4 * 4 + i4g : g4 * 4 + i4g + 1, :],
                                num_idxs=128,
                                num_idxs_reg=128,
                                elem_size=4 * C,
                                elem_step=2 * C,
                            )
                    else:
                        gd = gpool.tile([P, 4, 1024], F16, tag="gd", name=f"gd_{h}_{g4}_{kk}")
                        nc.gpsimd.dma_gather(
                            gd[:],
                            xpad_src,
                            idxs16[h][:, kk, g4 * 4 : (g4 + 1) * 4, :],
                            num_idxs=512,
                            num_idxs_reg=512,
                            elem_size=4 * C,
                            elem_step=2 * C,
                        )
                    ptile = [
                        psE.tile(
                            [P, 512], F32, tag=f"pt{ch}", name=f"pt{ch}_{h}_{g4}_{kk}"
                        )
                        for ch in range(2)
                    ]
                    for i4 in range(4):
                        st8 = g4 * 4 + i4
                        st = h * 8 + st8
                        # diag trick: dg4[:, q, :] = idn * w_q (per-sample
                        # diagonal); PE computes gd_q^T @ diag(w_q), i.e. the
                        # scaled transpose, with the scaling folded into the
                        # matmul rhs. Half the DVE elements of scaling the
                        # [128, 256] corners directly.
                        dg4 = sclp.tile([P, 4, P], F16, tag="dg4")
                        for q in range(4):
                            nc.vector.tensor_scalar_mul(
                                dg4[:, q, :],
                                t_idn16[:],
                                w4[:, st, kk, q : q + 1],
                            )
                        for ch in range(2):
                            for q in range(4):
                                nc.tensor.matmul(
                                    ptile[ch][:, i4 * P : (i4 + 1) * P],
                                    lhsT=gd[:, i4, q * C + ch * P : q * C + (ch + 1) * P],
                                    rhs=dg4[:, q, :],
                                    start=(q == 0),
                                    stop=(q == 3),
                                )
                    for ch in range(2):
                        t = kk * 2 + ch
                        # drain phase of the last quarter: split the copies
                        # across Act/DVE so the PSUM->sampled stage doesn't
                        # serialize the in-order queues
                        if last_q and kk >= 4 and ch == 1:
                            nc.vector.tensor_copy(sampled[:, t, :], ptile[ch][:])
                        else:
                            nc.scalar.copy(sampled[:, t, :], ptile[ch][:])
                    pend.append(kk)
                    if last_q:
                        # drain the leftover previous-quarter flushes, then
                        # stream this quarter's inline with delay 2 (1 for
                        # the final taps)
                        if prev is not None and prev[0]:
                            prev[1](prev[0].pop(0))
                            if kk >= 5 and prev[0]:
                                prev[1](prev[0].pop(0))
                        depth = 1 if kk >= KK - 2 else 2
                        while len(pend) > depth:
                            flush_gemm(pend.pop(0))
                    elif prev is not None and prev[0]:
                        # deferred GEMM of the previous quarter: one K-tile
                        # pair per tap; its data is a whole quarter old, so
                        # these never stall the PE queue — they fill its
                        # idle slots
                        prev[1](prev[0].pop(0))
                if prev is not None:
                    while prev[0]:
                        prev[1](prev[0].pop(0))
                    emit_out(prev[2], prev[3], prev[4], False)
                if last_q:
                    for dk in pend:
                        flush_gemm(dk)
                    emit_out(pso, h, g4, True)
                prev = (pend, flush_gemm, pso, h, g4)
                if debug_outputs:
                    nc.sync.dma_start(
                        out=dbg_smp.ap().rearrange(
                            "p (t q n) -> p t q n", q=4, n=512
                        )[:, :, h * 2 + g4, :],
                        in_=sampled[:],
                    )

    nc.compile()
    return nc


def host_prep(x, weight, bias, offset_w, offset_b):
    """Returns (in_maps list of 8 dicts, assemble fn)."""
    B = x.shape[0]
    xp = np.zeros((B, WP, WP, C), np.float16)
    xp[:, PAD : PAD + H, PAD : PAD + W, :] = x.transpose(0, 2, 3, 1)
    # pair table: row r = [pixel r | pixel r+72] so one 2KB fetch at rows
    # (r, r+1) yields all 4 bilinear corners.
    xpad2_b = []
    for b in range(B):
        flat = xp[b].reshape(NPIX, C)
        t2 = np.zeros((TBL2, 2 * C), np.float16)
        t2[: TBL2 - 1, 0:C] = flat[: TBL2 - 1]
        t2[: TBL2 - 1, C : 2 * C] = flat[72 : TBL2 - 1 + 72]
        xpad2_b.append(t2)
    # c-major padded image for the offset conv, per (b, hh): rows 32h+3 .. +37
    xcp = xp.transpose(0, 3, 1, 2).reshape(B, 2, P, WP, WP)  # [b, grp, 128, 72, 72]
    wre = np.ascontiguousarray(
        weight.reshape(C, 2, P, 3, 3).transpose(3, 4, 1, 2, 0).reshape(KK * 2, P, C)
    ).astype(np.float16)
    # t = kk*2 + ch ; value = offset_w[o, ch*128+i, ky, kx]; packed [P, 18*18]
    owre = np.ascontiguousarray(
        offset_w.reshape(18, 2, P, 3, 3).transpose(2, 3, 4, 1, 0).reshape(P, 18 * 18)
    ).astype(np.float16)
    idn16 = np.eye(P, dtype=np.float16)
    obcol = np.zeros((P, 1), np.float32)
    obcol[:18, 0] = offset_b
    # selector for the wrapped-idx matmuls: rsel[p, a, m] = 1 iff p == a*16 + m%16
    rsel = np.zeros((P, 8, P), np.float32)
    for a in range(8):
        for m in range(P):
            rsel[a * 16 + m % 16, a, m] = 1.0
    rsel = rsel.reshape(P, 8 * P)
    bcols = np.asarray(bias, np.float32).reshape(2, P).T.copy()  # [128, 2]

    base_all = []
    for hh in range(2):
        base = np.zeros((P, NT, 18), np.float32)
        p = np.arange(P)
        for st in range(NT):
            n = st * P + p
            ho = 32 * hh + n // W
            wo = n % W
            for kk in range(KK):
                ky, kx = kk // 3, kk % 3
                base[:, st, 2 * kk + 0] = ky + ho - 1 + PAD
                base[:, st, 2 * kk + 1] = kx + wo - 1 + PAD
        base_all.append(base.reshape(P, NT * 18))


    in_maps = []
    for core in range(8):
        b, hh = core // 2, core % 2
        in_maps.append(
            {
                "xpad2": xpad2_b[b],
                "xchw": np.ascontiguousarray(
                    xcp[b, :, :, 32 * hh + 3 : 32 * hh + 37, :].reshape(2, P, 34 * WP)
                ),
                "wre": wre,
                "owre": owre,
                "basep4": base_all[hh],
                "idn16": idn16,
                "idn18": np.eye(18, dtype=np.float32),
                "obcol": obcol,
                "bcols": bcols,
                "rsel": rsel,
            }
        )

    def assemble(results):
        y = np.empty((B, C, H, W), np.float32)
        for core in range(8):
            b, hh = core // 2, core % 2
            y[b, :, 32 * hh : 32 * (hh + 1), :] = (
                results[core]["out"].astype(np.float32).reshape(C, HO, W)
            )
        return y

    return in_maps, assemble


_CACHE = {}


def _maybe_reset_devices():
    # Clear any wedged accelerator state left by a previous crashed run.
    try:
        import ctypes
        import jax

        jax.devices()
        lib = ctypes.CDLL("/opt/axon/libaxon_pjrt.so")
        if hasattr(lib, "axon_reset"):
            lib.axon_reset.restype = ctypes.c_int64
            lib.axon_reset()
    except Exception:
        pass


def kernel(x, weight, bias, offset_w, offset_b, trace=False):
    if "nc" not in _CACHE:
        _maybe_reset_devices()
        _CACHE["nc"] = build()
    nc = _CACHE["nc"]
    in_maps, assemble = host_prep(
        np.asarray(x), np.asarray(weight), np.asarray(bias),
        np.asarray(offset_w), np.asarray(offset_b),
    )
    res = bass_utils.run_bass_kernel_spmd(
        nc, in_maps, core_ids=list(range(8)), trace=trace
    )
    out = assemble(res.results)
    _CACHE["last_exec_time_ns"] = res.exec_time_ns
    return out

